# revision 14
# baseline (speedup 1.0000x reference)
"""Trainium2 Bass kernel for a dense-MoE encoder layer (8 NeuronCores).

Sharding:
  - Attention: head-parallel (16 heads / 8 cores = 2 heads per core); the
    output-projection partial sums are AllReduced (bf16) per batch.
  - MoE: expert-parallel (8 experts / 8 cores = 1 expert per core); the
    gate-weighted expert outputs are ReduceScattered (bf16) per batch.

All matmuls run in bf16 with fp32 PSUM accumulation. Softmax skips the
max-subtraction (qk-norm bounds |score*scale| <= 8). Host-side prep does
layout/transpose/dtype conversion only. Emission is software-pipelined:
attn(b+1) is emitted between attn(b) and moe(b) so each batch's AllReduce
overlaps the next batch's attention and the previous batch's MoE.
"""

import numpy as np
import ml_dtypes

B, S, DIM, HEADS, DH = 4, 1024, 1024, 16, 64
E, HID = 8, 4096
NC = 8
HPC = HEADS // NC  # heads per core
SCALE = DH ** -0.5
EPS = 1e-5
NEG = -30000.0
TT = 256  # MoE token tile
NTT = S // TT
DC = DIM // 128  # 8 d-chunks
HC = HID // 128  # 32 hid-chunks

bf16np = ml_dtypes.bfloat16

_cache = {}


def _build():
    import concourse.mybir as mybir
    import concourse.tile as tile
    from concourse import bacc
    from concourse.bass import ts

    bf16 = mybir.dt.bfloat16
    f32 = mybir.dt.float32
    AF = mybir.ActivationFunctionType
    OP = mybir.AluOpType

    nc = bacc.Bacc(None, target_bir_lowering=False, debug=False, num_devices=NC)
    P = {}
    for name, shape, dt in [
        ("xT", [B, DC, 128, S], bf16),
        ("wqk", [DC, 128, 2 * 128], bf16),
        ("wv", [DC, 128, 128], bf16),
        ("wout", [DC, 128, 128], bf16),
        ("w1", [DC, 128, HID], bf16),
        ("w2", [HC, 128, DIM], bf16),
        ("gatew", [DC, 128, E], bf16),
        ("b1s", [128, HC], f32),
        ("b2s", [128, DC], f32),
        ("gateb", [E, 1], f32),
        ("qg", [128, 1], f32),
        ("kg", [128, 1], f32),
        ("cmaskT", [128, 128], f32),
        ("ones128k", [128, 1], bf16),
        ("onesB", [128, 64], f32),
        ("mmean", [128, 128], bf16),
        ("ones8", [E, 1], bf16),
        ("onehot", [E, 1], bf16),
        ("onesA", [2, 128], f32),
    ]:
        P[name] = nc.declare_dram_parameter(name, shape, dt, isOutput=False)
    out_e = nc.declare_dram_parameter("out", [B, 128, S], f32, isOutput=True)

    rg = [list(range(NC))]

    with tile.TileContext(nc, num_cores=NC) as tc:
        with (
            tc.tile_pool(name="wp", bufs=1) as wp,
            tc.tile_pool(name="sp", bufs=2) as sp,
            tc.tile_pool(name="pp", bufs=1, space="PSUM") as pp,
            tc.tile_pool(name="dp", bufs=1, space="DRAM") as dp,
        ):
            # ---- resident weights / constants ----
            w1_sb = wp.tile([128, DC, HID], bf16)
            w2_sb = wp.tile([128, HC, DIM], bf16)
            wqk_sb = wp.tile([128, DC, 256], bf16)
            wv_sb = wp.tile([128, DC, 128], bf16)
            wout_sb = wp.tile([128, DC, 128], bf16)
            gatew_sb = wp.tile([128, DC, E], bf16)
            for d in range(DC):
                nc.sync.dma_start(w1_sb[:, d, :], P["w1"][d])
                nc.sync.dma_start(wqk_sb[:, d, :], P["wqk"][d])
                nc.sync.dma_start(wv_sb[:, d, :], P["wv"][d])
                nc.sync.dma_start(wout_sb[:, d, :], P["wout"][d])
                nc.sync.dma_start(gatew_sb[:, d, :], P["gatew"][d])
            for h in range(HC):
                nc.sync.dma_start(w2_sb[:, h, :], P["w2"][h])
            b1_sb = wp.tile([128, HC], f32)
            b2_sb = wp.tile([128, DC], f32)
            gateb_sb = wp.tile([E, 1], f32)
            qg_sb = wp.tile([128, 1], f32)
            kg_sb = wp.tile([128, 1], f32)
            cmaskT_sb = wp.tile([128, 128], f32)
            ones128k_sb = wp.tile([128, 1], bf16)
            onesB_sb = wp.tile([128, 64], f32)
            mmean_sb = wp.tile([128, 128], bf16)
            ones8_sb = wp.tile([E, 1], bf16)
            onehot_sb = wp.tile([E, 1], bf16)
            onesA_sb = wp.tile([2, 128], f32)
            eps_sb = wp.tile([128, 1], f32)
            for nm, t in [
                ("b1s", b1_sb), ("b2s", b2_sb), ("gateb", gateb_sb),
                ("qg", qg_sb), ("kg", kg_sb), ("cmaskT", cmaskT_sb),
                ("ones128k", ones128k_sb), ("onesB", onesB_sb),
                ("mmean", mmean_sb),
                ("ones8", ones8_sb), ("onehot", onehot_sb), ("onesA", onesA_sb),
            ]:
                nc.sync.dma_start(t, P[nm][:, :])
            nc.vector.memset(eps_sb, EPS)

            arin, arout, rsin, rsout = [], [], [], []
            for b in range(B):
                arin.append(dp.tile([DC, 128, S], bf16, name=f"arin{b}"))
                arout.append(dp.tile([DC, 128, S], bf16, name=f"arout{b}",
                                     addr_space="Shared"))
                rsin.append(dp.tile([DC, 128, S], bf16, name=f"rsin{b}"))
                rsout.append(dp.tile([128, S], bf16, name=f"rsout{b}"))

            # PSUM tags (8 banks total):
            #   y: [128,8,256] f32 = 4 banks, bufs=1
            #   h: [128,2,256] f32 = 1 bank, bufs=1 (manual hc%2 rotation)
            #   ws: [128,512] f32 = 1 bank, bufs=2 (all transient matmul outs)
            #   pv: [64,128] f32 = 1 bank, bufs=1 (attention PV accumulation)

            def emit_attention(b):
                # ---- qkv projection (feature-major q,k) + qk-norm ----
                qkT = sp.tile([128, 2, S], bf16, tag="qkT", bufs=1,
                              name=f"qkT{b}")
                for nch in range(2):
                    qk_ps = []
                    for m in range(2):
                        qk_ps.append(pp.tile([128, 512], f32, tag="ws", bufs=2,
                                             name=f"qkps{b}_{nch}_{m}"))
                    for d in range(DC):
                        xt = sp.tile([128, 512], bf16, tag="xt", bufs=2,
                                     name=f"xt{b}_{nch}_{d}")
                        nc.sync.dma_start(xt, P["xT"][b, d, :, ts(nch, 512)])
                        for m in range(2):
                            nc.tensor.matmul(
                                qk_ps[m], wqk_sb[:, d, ts(m, 128)], xt,
                                start=(d == 0), stop=(d == DC - 1))
                    for m in range(2):
                        nc.scalar.copy(qkT[:, m, ts(nch, 512)], qk_ps[m])
                    for m in range(2):
                        qraw = qkT[:, m, ts(nch, 512)]
                        mu_ps = pp.tile([128, 512], f32, tag="ws", bufs=2,
                                        name=f"mups{b}_{nch}_{m}")
                        nc.tensor.matmul(mu_ps, mmean_sb, qraw,
                                         start=True, stop=True)
                        sub = sp.tile([128, 512], bf16, tag="sub",
                                      name=f"sub{b}_{nch}_{m}")
                        nc.vector.tensor_tensor(sub, qraw, mu_ps, OP.subtract)
                        sq = sp.tile([128, 512], bf16, tag="sq",
                                     name=f"sq{b}_{nch}_{m}")
                        nc.vector.tensor_mul(sq, sub, sub)
                        var_ps = pp.tile([128, 512], f32, tag="ws", bufs=2,
                                         name=f"varps{b}_{nch}_{m}")
                        nc.tensor.matmul(var_ps, mmean_sb, sq,
                                         start=True, stop=True)
                        rstd = sp.tile([128, 512], f32, tag="rstd",
                                       name=f"rstd{b}_{nch}_{m}")
                        nc.scalar.activation(rstd, var_ps, AF.Sqrt,
                                             bias=eps_sb, scale=1.0)
                        nc.vector.reciprocal(rstd, rstd)
                        g_ap = qg_sb if m == 0 else kg_sb
                        nc.vector.scalar_tensor_tensor(
                            qkT[:, m, ts(nch, 512)], sub, g_ap, rstd,
                            op0=OP.mult, op1=OP.mult)
                # ---- v (token-major) ----
                v_sb = sp.tile([128, DC, 128], bf16, tag="v_sb",
                               name=f"vsb{b}")
                for vg in range(2):  # two groups of 4 token-chunks
                    vq = pp.tile([128, 4, 128], f32, tag="ws", bufs=2,
                                 name=f"vq{b}_{vg}")
                    for tq in range(4):
                        tcn = vg * 4 + tq
                        for d in range(DC):
                            xtv = sp.tile([128, 128], bf16, tag="xtv", bufs=4,
                                          name=f"xtv{b}_{tcn}_{d}")
                            nc.sync.dma_start(
                                xtv, P["xT"][b, d, :, ts(tcn, 128)])
                            nc.tensor.matmul(
                                vq[:, tq, :], xtv, wv_sb[:, d, :],
                                start=(d == 0), stop=(d == DC - 1))
                        nc.scalar.copy(v_sb[:, tcn, :], vq[:, tq, :])
                # ---- causal attention, 2 heads ----
                oTh = sp.tile([128, S], bf16, tag="oTh", bufs=1,
                              name=f"oTh{b}")
                for h in range(HPC):
                    hsl = slice(64 * h, 64 * h + 64)
                    dsl = slice(64 * h, 64 * h + 1)
                    for qc in range(8):
                        # pvt holds the unnormalized PV output in cols 0:128
                        # and the softmax denominator row in cols 128:256
                        pvt = pp.tile([128, 256], f32, tag="pv", bufs=1,
                                      name=f"pvt{b}_{h}_{qc}")
                        for kc in range(qc + 1):
                            scT = pp.tile([128, 128], f32, tag="ws", bufs=2,
                                          name=f"scT{b}_{h}_{qc}_{kc}")
                            nc.tensor.matmul(
                                scT,
                                qkT[hsl, 1, ts(kc, 128)],
                                qkT[hsl, 0, ts(qc, 128)],
                                start=True, stop=True)
                            if kc == qc:
                                nc.vector.tensor_add(scT, scT, cmaskT_sb)
                            exk = sp.tile([128, 128], bf16, tag="exk", bufs=4,
                                          name=f"exk{b}_{h}_{qc}_{kc}")
                            nc.scalar.activation(exk, scT, AF.Exp, scale=SCALE)
                            nc.tensor.matmul(
                                pvt[hsl, 0:128], v_sb[:, kc, hsl], exk,
                                start=(kc == 0), stop=(kc == qc))
                            nc.tensor.matmul(
                                pvt[dsl, 128:256], ones128k_sb, exk,
                                start=False, stop=False, skip_group_check=True)
                        rec_sb = sp.tile([128, 128], f32, tag="rec_sb",
                                         bufs=2, name=f"rcs{b}_{h}_{qc}")
                        nc.vector.reciprocal(rec_sb[dsl, :],
                                             pvt[dsl, 128:256])
                        rb = pp.tile([128, 128], f32, tag="ws", bufs=2,
                                     name=f"rb{b}_{h}_{qc}")
                        nc.tensor.matmul(rb[hsl, :], onesB_sb[dsl, :],
                                         rec_sb[dsl, :],
                                         start=True, stop=True)
                        rb_sb = sp.tile([128, 128], f32, tag="rb_sb",
                                        bufs=2, name=f"rbs{b}_{h}_{qc}")
                        nc.scalar.copy(rb_sb[hsl, :], rb[hsl, :])
                        nc.vector.tensor_mul(oTh[hsl, ts(qc, 128)],
                                             pvt[hsl, 0:128], rb_sb[hsl, :])
                # ---- out-projection partial -> arin -> AllReduce ----
                for mc in range(DC):
                    for nch in range(2):
                        wo_ps = pp.tile([128, 512], f32, tag="ws", bufs=2,
                                        name=f"wops{b}_{mc}_{nch}")
                        nc.tensor.matmul(wo_ps, wout_sb[:, mc, :],
                                         oTh[:, ts(nch, 512)],
                                         start=True, stop=True)
                        wo_bf = sp.tile([128, 512], bf16, tag="wo_bf", bufs=2,
                                        name=f"wobf{b}_{mc}_{nch}")
                        nc.vector.tensor_copy(wo_bf, wo_ps)
                        nc.sync.dma_start(arin[b][mc, :, ts(nch, 512)], wo_bf)
                nc.gpsimd.collective_compute(
                    "AllReduce", OP.add, replica_groups=rg,
                    ins=[arin[b].opt()], outs=[arout[b].opt()])

            def emit_moe(b):
                oT = sp.tile([128, DC, S], bf16, tag="oT", bufs=1,
                             name=f"oT{b}")
                for d in range(DC):
                    nc.sync.dma_start(oT[:, d, :], arout[b][d])
                # ---- gates ----
                g_bc = sp.tile([128, S], bf16, tag="g_bc", bufs=1,
                               name=f"gbc{b}")
                for nch in range(2):
                    lg = pp.tile([E, 512], f32, tag="ws", bufs=2,
                                 name=f"lg{b}_{nch}")
                    for d in range(DC):
                        nc.tensor.matmul(lg, gatew_sb[:, d, :],
                                         oT[:, d, ts(nch, 512)],
                                         start=(d == 0), stop=(d == DC - 1))
                    expT = sp.tile([E, 512], bf16, tag="expT",
                                   name=f"expT{b}_{nch}")
                    nc.scalar.activation(expT, lg, AF.Exp, bias=gateb_sb,
                                         scale=1.0)
                    den_ps = pp.tile([1, 512], f32, tag="ws", bufs=2,
                                     name=f"denps{b}_{nch}")
                    nc.tensor.matmul(den_ps, ones8_sb, expT,
                                     start=True, stop=True)
                    sel_ps = pp.tile([1, 512], f32, tag="ws", bufs=2,
                                     name=f"selps{b}_{nch}")
                    nc.tensor.matmul(sel_ps, onehot_sb, expT,
                                     start=True, stop=True)
                    den_sb = sp.tile([1, 512], f32, tag="den_sb",
                                     name=f"densb{b}_{nch}")
                    nc.scalar.copy(den_sb, den_ps)
                    sel_sb = sp.tile([1, 512], f32, tag="sel_sb",
                                     name=f"selsb{b}_{nch}")
                    nc.scalar.copy(sel_sb, sel_ps)
                    # broadcast denominator and selected-expert rows to 128
                    # partitions via K=1 matmuls, then g = sel * (1/den)
                    gbd = pp.tile([128, 512], f32, tag="ws", bufs=2,
                                  name=f"gbd{b}_{nch}")
                    nc.tensor.matmul(gbd, onesA_sb[0:1, :], den_sb,
                                     start=True, stop=True)
                    gbs = pp.tile([128, 512], f32, tag="ws", bufs=2,
                                  name=f"gbs{b}_{nch}")
                    nc.tensor.matmul(gbs, onesA_sb[0:1, :], sel_sb,
                                     start=True, stop=True)
                    recd = sp.tile([128, 512], f32, tag="recd", bufs=1,
                                   name=f"recd{b}_{nch}")
                    nc.vector.reciprocal(recd, gbd)
                    nc.vector.tensor_mul(g_bc[:, ts(nch, 512)], gbs, recd)
                # ---- FFN expert ----
                for tt in range(NTT):
                    y_ps = pp.tile([128, DC, TT], f32, tag="y", bufs=1,
                                   name=f"yps{b}_{tt}")
                    h_ps = pp.tile([128, 2, TT], f32, tag="h", bufs=1,
                                   name=f"hps{b}_{tt}")
                    for hc in range(HC):
                        hslot = h_ps[:, hc % 2, :]
                        for d in range(DC):
                            nc.tensor.matmul(
                                hslot,
                                w1_sb[:, d, ts(hc, 128)],
                                oT[:, d, ts(tt, TT)],
                                start=(d == 0), stop=(d == DC - 1))
                        hT = sp.tile([128, TT], bf16, tag="hT", bufs=3,
                                     name=f"hT{b}_{tt}_{hc}")
                        nc.scalar.activation(hT, hslot, AF.Gelu_apprx_tanh,
                                             bias=b1_sb[:, hc:hc + 1],
                                             scale=1.0)
                        for d2 in range(DC):
                            nc.tensor.matmul(
                                y_ps[:, d2, :],
                                w2_sb[:, hc, ts(d2, 128)],
                                hT,
                                start=(hc == 0 and d2 % 2 == 0),
                                stop=(hc == HC - 1 and d2 % 2 == 1))
                    for d2 in range(DC):
                        y_bf = sp.tile([128, TT], bf16, tag="y_bf", bufs=3,
                                       name=f"ybf{b}_{tt}_{d2}")
                        nc.vector.scalar_tensor_tensor(
                            y_bf, y_ps[:, d2, :], b2_sb[:, d2:d2 + 1],
                            g_bc[:, ts(tt, TT)],
                            op0=OP.add, op1=OP.mult)
                        nc.sync.dma_start(rsin[b][d2, :, ts(tt, TT)], y_bf)
                nc.gpsimd.collective_compute(
                    "ReduceScatter", OP.add, replica_groups=rg,
                    ins=[rsin[b].opt()], outs=[rsout[b].opt()])
                for nch in range(2):
                    ob_bf = sp.tile([128, 512], bf16, tag="ob_bf", bufs=1,
                                    name=f"obbf{b}_{nch}")
                    nc.sync.dma_start(ob_bf, rsout[b][:, ts(nch, 512)])
                    ob = sp.tile([128, 512], f32, tag="ob", bufs=1,
                                 name=f"ob{b}_{nch}")
                    nc.vector.tensor_copy(ob, ob_bf)
                    nc.sync.dma_start(out_e[b, :, ts(nch, 512)], ob)

            # software-pipelined emission: attn runs one batch ahead of moe
            emit_attention(0)
            for b in range(1, B):
                emit_attention(b)
                emit_moe(b - 1)
            emit_moe(B - 1)

    nc.compile()
    return nc


def _prep_inputs(inputs):
    """Host-side shard prep: slice/transpose/cast per core."""
    f32 = np.float32

    def b(x):
        return np.ascontiguousarray(x).astype(bf16np)

    x = inputs["x"].astype(f32)
    w_qkv = inputs["w_qkv"].astype(f32)
    w_out = inputs["w_out"].astype(f32)
    qn_g, kn_g = inputs["qn_g"].astype(f32), inputs["kn_g"].astype(f32)
    gate_w, gate_b = inputs["gate_w"].astype(f32), inputs["gate_b"].astype(f32)
    w1, b1, w2, b2 = (inputs["w1"].astype(f32), inputs["b1"].astype(f32),
                      inputs["w2"].astype(f32), inputs["b2"].astype(f32))

    xT = b(x.transpose(0, 2, 1).reshape(B, DC, 128, S))
    gatewr = b(gate_w.reshape(DC, 128, E))
    ii, jj = np.meshgrid(np.arange(128), np.arange(128), indexing="ij")
    cmaskT = np.where(ii <= jj, 0.0, NEG).astype(f32)
    mmean = b(np.where(ii // 64 == jj // 64, 1.0 / 64, 0.0).astype(f32))
    onesA = np.ones((2, 128), f32)
    qg = np.tile(qn_g, 2).reshape(128, 1).astype(f32)
    kg = np.tile(kn_g, 2).reshape(128, 1).astype(f32)
    gateb = gate_b.reshape(E, 1).astype(f32)

    in_maps = []
    for c in range(NC):
        h0, h1 = HPC * c, HPC * c + 1
        cs = np.r_[h0 * 64:(h0 + 1) * 64, h1 * 64:(h1 + 1) * 64]
        wqk_c = np.concatenate(
            [w_qkv[:, cs], w_qkv[:, DIM + cs]], axis=1)          # [1024,256]
        wv_c = w_qkv[:, 2 * DIM + cs]                            # [1024,128]
        wout_c = w_out[cs, :]                                    # [128,1024]
        onehot = np.zeros((E, 1), bf16np)
        onehot[c, 0] = 1.0
        in_maps.append({
            "xT": xT,
            "wqk": b(wqk_c.reshape(DC, 128, 256)),
            "wv": b(wv_c.reshape(DC, 128, 128)),
            "wout": b(wout_c.reshape(128, DC, 128).transpose(1, 0, 2)),
            "w1": b(w1[c].reshape(DC, 128, HID)),
            "w2": b(w2[c].reshape(HC, 128, DIM)),
            "gatew": gatewr,
            "b1s": np.ascontiguousarray(b1[c].reshape(HC, 128).T).astype(f32),
            "b2s": np.ascontiguousarray(b2[c].reshape(DC, 128).T).astype(f32),
            "gateb": gateb,
            "qg": qg,
            "kg": kg,
            "cmaskT": cmaskT,
            "ones128k": np.ones((128, 1), bf16np),
            "onesB": np.ones((128, 64), f32),
            "mmean": mmean,
            "ones8": np.ones((E, 1), bf16np),
            "onehot": onehot,
            "onesA": onesA,
        })
    return in_maps


def kernel(**inputs):
    from concourse.bass_utils import run_bass_kernel_spmd

    if "nc" not in _cache:
        _cache["nc"] = _build()
    nc = _cache["nc"]
    in_maps = _prep_inputs(inputs)
    res = run_bass_kernel_spmd(nc, in_maps, core_ids=list(range(NC)))
    full = np.empty((B, S, DIM), np.float32)
    for c in range(NC):
        full[:, :, 128 * c:128 * (c + 1)] = (
            res.results[c]["out"].transpose(0, 2, 1))
    return full


# revision 22
# speedup vs baseline: 1.0185x; 1.0185x over previous
"""Trainium2 Bass kernel for a dense-MoE encoder layer (8 NeuronCores).

Sharding:
  - Attention: head-parallel (16 heads / 8 cores = 2 heads per core); the
    output-projection partial sums are AllReduced (bf16) per batch.
  - MoE: expert-parallel (8 experts / 8 cores = 1 expert per core); the
    gate-weighted expert outputs are ReduceScattered (bf16) per batch.

All matmuls run in bf16 with fp32 PSUM accumulation. Softmax skips the
max-subtraction (qk-norm bounds |score*scale| <= 8). Host-side prep does
layout/transpose/dtype conversion only. Emission is software-pipelined:
attn(b+1) is emitted between attn(b) and moe(b) so each batch's AllReduce
overlaps the next batch's attention and the previous batch's MoE.
"""

import numpy as np
import ml_dtypes

B, S, DIM, HEADS, DH = 4, 1024, 1024, 16, 64
E, HID = 8, 4096
NC = 8
HPC = HEADS // NC  # heads per core
SCALE = DH ** -0.5
EPS = 1e-5
NEG = -30000.0
TT = 256  # MoE token tile
NTT = S // TT
DC = DIM // 128  # 8 d-chunks
HC = HID // 128  # 32 hid-chunks

bf16np = ml_dtypes.bfloat16

_cache = {}


def _build(debug_attn=False):
    import concourse.mybir as mybir
    import concourse.tile as tile
    from concourse import bacc
    from concourse.bass import ts

    bf16 = mybir.dt.bfloat16
    f32 = mybir.dt.float32
    AF = mybir.ActivationFunctionType
    OP = mybir.AluOpType

    import bass_rust

    def dep(a, b, why):
        # a must run after b (same-engine ordering for PSUM zero-region tricks)
        bass_rust.add_dep_helper(a.ins, b.ins, reason=why)

    nc = bacc.Bacc(None, target_bir_lowering=False, debug=False, num_devices=NC)
    P = {}
    for name, shape, dt in [
        ("xT", [B, DC, 128, S], bf16),
        ("wqk", [DC, 128, 2 * 128], bf16),
        ("wv", [DC, 128, 128], bf16),
        ("wout", [DC, 128, 128], bf16),
        ("w1", [DC, 128, HID], bf16),
        ("w2", [HC, 128, DIM], bf16),
        ("gatew", [DC, 128, E], bf16),
        ("b1s", [128, HC], f32),
        ("b2s", [128, DC], f32),
        ("gateb", [E, 1], f32),
        ("qg", [128, 1], f32),
        ("kg", [128, 1], f32),
        ("cmaskT2a", [128, 256], f32),
        ("cmaskT2b", [128, 256], f32),
        ("ones128k", [128, 1], bf16),
        ("onesB", [128, 64], f32),
        ("mmean", [128, 128], bf16),
        ("ones8", [E, 1], bf16),
        ("onehot", [E, 1], bf16),
        ("onesA", [2, 128], f32),
    ]:
        P[name] = nc.declare_dram_parameter(name, shape, dt, isOutput=False)
    out_e = nc.declare_dram_parameter("out", [B, 128, S], f32, isOutput=True)

    rg = [list(range(NC))]

    with tile.TileContext(nc, num_cores=NC) as tc:
        with (
            tc.tile_pool(name="wp", bufs=1) as wp,
            tc.tile_pool(name="sp", bufs=2) as sp,
            tc.tile_pool(name="pp", bufs=1, space="PSUM") as pp,
            tc.tile_pool(name="dp", bufs=1, space="DRAM") as dp,
        ):
            # ---- resident weights / constants ----
            w1_sb = wp.tile([128, DC, HID], bf16)
            w2_sb = wp.tile([128, HC, DIM], bf16)
            wqk_sb = wp.tile([128, DC, 256], bf16)
            wv_sb = wp.tile([128, DC, 128], bf16)
            wout_sb = wp.tile([128, DC, 128], bf16)
            gatew_sb = wp.tile([128, DC, E], bf16)
            for d in range(DC):
                nc.sync.dma_start(w1_sb[:, d, :], P["w1"][d])
                nc.sync.dma_start(wqk_sb[:, d, :], P["wqk"][d])
                nc.sync.dma_start(wv_sb[:, d, :], P["wv"][d])
                nc.sync.dma_start(wout_sb[:, d, :], P["wout"][d])
                nc.sync.dma_start(gatew_sb[:, d, :], P["gatew"][d])
            for h in range(HC):
                nc.sync.dma_start(w2_sb[:, h, :], P["w2"][h])
            b1_sb = wp.tile([128, HC], f32)
            b2_sb = wp.tile([128, DC], f32)
            gateb_sb = wp.tile([E, 1], f32)
            qg_sb = wp.tile([128, 1], f32)
            kg_sb = wp.tile([128, 1], f32)
            cm2a_sb = wp.tile([128, 256], f32)
            cm2b_sb = wp.tile([128, 256], f32)
            ones128k_sb = wp.tile([128, 1], bf16)
            onesB_sb = wp.tile([128, 64], f32)
            mmean_sb = wp.tile([128, 128], bf16)
            ones8_sb = wp.tile([E, 1], bf16)
            onehot_sb = wp.tile([E, 1], bf16)
            onesA_sb = wp.tile([2, 128], f32)
            eps_sb = wp.tile([128, 1], f32)
            for nm, t in [
                ("b1s", b1_sb), ("b2s", b2_sb), ("gateb", gateb_sb),
                ("qg", qg_sb), ("kg", kg_sb),
                ("cmaskT2a", cm2a_sb), ("cmaskT2b", cm2b_sb),
                ("ones128k", ones128k_sb), ("onesB", onesB_sb),
                ("mmean", mmean_sb),
                ("ones8", ones8_sb), ("onehot", onehot_sb), ("onesA", onesA_sb),
            ]:
                nc.sync.dma_start(t, P[nm][:, :])
            nc.vector.memset(eps_sb, EPS)

            arin, arout, rsin, rsout = [], [], [], []
            for b in range(B):
                arin.append(dp.tile([DC, 128, S], bf16, name=f"arin{b}"))
                arout.append(dp.tile([DC, 128, S], bf16, name=f"arout{b}",
                                     addr_space="Shared"))
                rsin.append(dp.tile([DC, 128, S], bf16, name=f"rsin{b}"))
                rsout.append(dp.tile([128, S], bf16, name=f"rsout{b}"))

            # PSUM tags (8 banks total):
            #   y: [128,8,256] f32 = 4 banks, bufs=1
            #   h: [128,256] f32 = 1 bank, bufs=2 (double-buffered h GEMM)
            #   ws: [128,512] f32 = 1 bank, bufs=1 (all transient matmul outs)
            #   pv: [128,512] f32 = 1 bank, bufs=1 (PV accum + softmax denom)

            def emit_attention(b):
                # ---- qkv projection (feature-major q,k) + qk-norm ----
                qkT = sp.tile([128, 2, S], bf16, tag="qkT", bufs=1,
                              name=f"qkT{b}")
                for nch in range(2):
                    for m in range(2):
                        qk_ps = pp.tile([128, 512], f32, tag="ws", bufs=1,
                                        name=f"qkps{b}_{nch}_{m}")
                        for d in range(DC):
                            xt = sp.tile([128, 512], bf16, tag="xt", bufs=2,
                                         name=f"xt{b}_{nch}_{m}_{d}")
                            nc.sync.dma_start(xt,
                                              P["xT"][b, d, :, ts(nch, 512)])
                            nc.tensor.matmul(
                                qk_ps, wqk_sb[:, d, ts(m, 128)], xt,
                                start=(d == 0), stop=(d == DC - 1))
                        nc.scalar.copy(qkT[:, m, ts(nch, 512)], qk_ps)
                    for m in range(2):
                        qraw = qkT[:, m, ts(nch, 512)]
                        mu_ps = pp.tile([128, 512], f32, tag="ws", bufs=1,
                                        name=f"mups{b}_{nch}_{m}")
                        nc.tensor.matmul(mu_ps, mmean_sb, qraw,
                                         start=True, stop=True)
                        sub = sp.tile([128, 512], bf16, tag="sub",
                                      name=f"sub{b}_{nch}_{m}")
                        nc.vector.tensor_tensor(sub, qraw, mu_ps, OP.subtract)
                        sq = sp.tile([128, 512], bf16, tag="sq",
                                     name=f"sq{b}_{nch}_{m}")
                        nc.vector.tensor_mul(sq, sub, sub)
                        var_ps = pp.tile([128, 512], f32, tag="ws", bufs=1,
                                         name=f"varps{b}_{nch}_{m}")
                        nc.tensor.matmul(var_ps, mmean_sb, sq,
                                         start=True, stop=True)
                        rstd = sp.tile([128, 512], f32, tag="rstd",
                                       name=f"rstd{b}_{nch}_{m}")
                        nc.scalar.activation(rstd, var_ps, AF.Sqrt,
                                             bias=eps_sb, scale=1.0)
                        nc.vector.reciprocal(rstd, rstd)
                        g_ap = qg_sb if m == 0 else kg_sb
                        nc.vector.scalar_tensor_tensor(
                            qkT[:, m, ts(nch, 512)], sub, g_ap, rstd,
                            op0=OP.mult, op1=OP.mult)
                # ---- v (token-major) ----
                v_sb = sp.tile([128, DC, 128], bf16, tag="v_sb",
                               name=f"vsb{b}")
                for vg in range(2):  # two groups of 4 token-chunks
                    vq = pp.tile([128, 4, 128], f32, tag="ws", bufs=1,
                                 name=f"vq{b}_{vg}")
                    vfirst, vlasts = None, []
                    for tq in range(4):
                        tcn = vg * 4 + tq
                        for d in range(DC):
                            xtv = sp.tile([128, 128], bf16, tag="xtv", bufs=4,
                                          name=f"xtv{b}_{tcn}_{d}")
                            nc.sync.dma_start(
                                xtv, P["xT"][b, d, :, ts(tcn, 128)])
                            mm = nc.tensor.matmul(
                                vq[:, tq, :], xtv, wv_sb[:, d, :],
                                start=(tq == 0 and d == 0),
                                stop=(tq == 3 and d == DC - 1),
                                skip_group_check=(tq != 0))
                            if tq == 0 and d == 0:
                                vfirst = mm
                            elif d == 0:
                                dep(mm, vfirst, "v zero-region after start")
                            if d == DC - 1 and tq < 3:
                                vlasts.append(mm)
                            if tq == 3 and d == DC - 1:
                                for vl in vlasts:
                                    dep(mm, vl, "v stop after all groups")
                        nc.scalar.copy(v_sb[:, tcn, :], vq[:, tq, :])
                # ---- causal attention, 2 heads ----
                oTh = sp.tile([128, S], bf16, tag="oTh", bufs=1,
                              name=f"oTh{b}")
                for qp in range(4):  # 256-query pairs, both heads share pvt
                    pvt = pp.tile([128, 512], f32, tag="pv", bufs=1,
                                  name=f"pvt{b}_{qp}")
                    nkc = 2 * qp + 2
                    for h in range(HPC):
                        hsl = slice(64 * h, 64 * h + 64)
                        dsl = slice(64 * h, 64 * h + 1)
                        pvfirst, pvden = None, None
                        for kc in range(nkc):
                            scT = pp.tile([128, 256], f32, tag="ws", bufs=1,
                                          name=f"scT{b}_{qp}_{h}_{kc}")
                            nc.tensor.matmul(
                                scT,
                                qkT[hsl, 1, ts(kc, 128)],
                                qkT[hsl, 0, ts(qp, 256)],
                                start=True, stop=True)
                            if kc == nkc - 2:
                                nc.vector.tensor_add(scT, scT, cm2a_sb)
                            elif kc == nkc - 1:
                                nc.vector.tensor_add(scT, scT, cm2b_sb)
                            exk = sp.tile([128, 256], bf16, tag="exk", bufs=4,
                                          name=f"exk{b}_{qp}_{h}_{kc}")
                            nc.scalar.activation(exk, scT, AF.Exp, scale=SCALE)
                            pvmm = nc.tensor.matmul(
                                pvt[hsl, 0:256], v_sb[:, kc, hsl], exk,
                                start=(kc == 0), stop=(kc == nkc - 1))
                            if kc == 0:
                                pvfirst = pvmm
                            if kc == nkc - 1 and pvden is not None:
                                dep(pvmm, pvden, "pv stop after last den")
                            dmm = nc.tensor.matmul(
                                pvt[dsl, 256:512], ones128k_sb, exk,
                                start=False, stop=False, skip_group_check=True)
                            if kc == 0:
                                dep(dmm, pvfirst, "den zero after pv start")
                            pvden = dmm
                    for h in range(HPC):
                        hsl = slice(64 * h, 64 * h + 64)
                        dsl = slice(64 * h, 64 * h + 1)
                        rec_sb = sp.tile([128, 256], f32, tag="rec_sb",
                                         bufs=2, name=f"rcs{b}_{qp}_{h}")
                        nc.vector.reciprocal(rec_sb[dsl, :],
                                             pvt[dsl, 256:512])
                        rb = pp.tile([128, 256], f32, tag="ws", bufs=1,
                                     name=f"rb{b}_{qp}_{h}")
                        nc.tensor.matmul(rb[hsl, :], onesB_sb[dsl, :],
                                         rec_sb[dsl, :],
                                         start=True, stop=True)
                        rb_sb = sp.tile([128, 256], f32, tag="rb_sb",
                                        bufs=2, name=f"rbs{b}_{qp}_{h}")
                        nc.scalar.copy(rb_sb[hsl, :], rb[hsl, :])
                        nc.vector.tensor_mul(oTh[hsl, ts(qp, 256)],
                                             pvt[hsl, 0:256], rb_sb[hsl, :])
                # ---- out-projection partial -> arin -> AllReduce ----
                for mc in range(DC):
                    for nch in range(2):
                        wo_ps = pp.tile([128, 512], f32, tag="ws", bufs=1,
                                        name=f"wops{b}_{mc}_{nch}")
                        nc.tensor.matmul(wo_ps, wout_sb[:, mc, :],
                                         oTh[:, ts(nch, 512)],
                                         start=True, stop=True)
                        wo_bf = sp.tile([128, 512], bf16, tag="wo_bf", bufs=2,
                                        name=f"wobf{b}_{mc}_{nch}")
                        nc.vector.tensor_copy(wo_bf, wo_ps)
                        nc.sync.dma_start(arin[b][mc, :, ts(nch, 512)], wo_bf)
                nc.gpsimd.collective_compute(
                    "AllReduce", OP.add, replica_groups=rg,
                    ins=[arin[b].opt()], outs=[arout[b].opt()])

            def emit_moe(b):
                oT = sp.tile([128, DC, S], bf16, tag="oT", bufs=1,
                             name=f"oT{b}")
                for d in range(DC):
                    nc.sync.dma_start(oT[:, d, :], arout[b][d])
                pass
                # ---- gates ----
                g_bc = sp.tile([128, S], bf16, tag="g_bc", bufs=1,
                               name=f"gbc{b}")
                for nch in range(2):
                    lg = pp.tile([E, 512], f32, tag="ws", bufs=1,
                                 name=f"lg{b}_{nch}")
                    for d in range(DC):
                        nc.tensor.matmul(lg, gatew_sb[:, d, :],
                                         oT[:, d, ts(nch, 512)],
                                         start=(d == 0), stop=(d == DC - 1))
                    expT = sp.tile([E, 512], bf16, tag="expT",
                                   name=f"expT{b}_{nch}")
                    nc.scalar.activation(expT, lg, AF.Exp, bias=gateb_sb,
                                         scale=1.0)
                    den_ps = pp.tile([1, 512], f32, tag="ws", bufs=1,
                                     name=f"denps{b}_{nch}")
                    nc.tensor.matmul(den_ps, ones8_sb, expT,
                                     start=True, stop=True)
                    sel_ps = pp.tile([1, 512], f32, tag="ws", bufs=1,
                                     name=f"selps{b}_{nch}")
                    nc.tensor.matmul(sel_ps, onehot_sb, expT,
                                     start=True, stop=True)
                    den_sb = sp.tile([1, 512], f32, tag="den_sb",
                                     name=f"densb{b}_{nch}")
                    nc.scalar.copy(den_sb, den_ps)
                    sel_sb = sp.tile([1, 512], f32, tag="sel_sb",
                                     name=f"selsb{b}_{nch}")
                    nc.scalar.copy(sel_sb, sel_ps)
                    # broadcast denominator and selected-expert rows to 128
                    # partitions via K=1 matmuls, then g = sel * (1/den)
                    gbd = pp.tile([128, 512], f32, tag="ws", bufs=1,
                                  name=f"gbd{b}_{nch}")
                    nc.tensor.matmul(gbd, onesA_sb[0:1, :], den_sb,
                                     start=True, stop=True)
                    gbs = pp.tile([128, 512], f32, tag="ws", bufs=1,
                                  name=f"gbs{b}_{nch}")
                    nc.tensor.matmul(gbs, onesA_sb[0:1, :], sel_sb,
                                     start=True, stop=True)
                    recd = sp.tile([128, 512], f32, tag="recd", bufs=1,
                                   name=f"recd{b}_{nch}")
                    nc.vector.reciprocal(recd, gbd)
                    nc.vector.tensor_mul(g_bc[:, ts(nch, 512)], gbs, recd)
                # ---- FFN expert ----
                for tt in range(NTT):
                    y_ps = pp.tile([128, DC, TT], f32, tag="y", bufs=1,
                                   name=f"yps{b}_{tt}")

                    def emit_h(hc, tt=tt):
                        hp = pp.tile([128, TT], f32, tag="h", bufs=2,
                                     name=f"hps{b}_{tt}_{hc}")
                        for d in range(DC):
                            nc.tensor.matmul(
                                hp,
                                w1_sb[:, d, ts(hc, 128)],
                                oT[:, d, ts(tt, TT)],
                                start=(d == 0), stop=(d == DC - 1))
                        return hp

                    hp = emit_h(0)
                    for hc in range(HC):
                        hT = sp.tile([128, TT], bf16, tag="hT", bufs=3,
                                     name=f"hT{b}_{tt}_{hc}")
                        nc.scalar.activation(hT, hp, AF.Gelu_apprx_tanh,
                                             bias=b1_sb[:, hc:hc + 1],
                                             scale=1.0)
                        if hc + 1 < HC:
                            hp = emit_h(hc + 1)
                        for d2 in range(DC):
                            ymm = nc.tensor.matmul(
                                y_ps[:, d2, :],
                                w2_sb[:, hc, ts(d2, 128)],
                                hT,
                                start=(hc == 0 and d2 % 2 == 0),
                                stop=(hc == HC - 1 and d2 % 2 == 1),
                                skip_group_check=(d2 % 2 == 1))
                            if hc == 0 and d2 % 2 == 0:
                                ylast = ymm
                            elif hc == 0 and d2 % 2 == 1:
                                dep(ymm, ylast, "y odd zero after even start")
                            if hc == HC - 1 and d2 % 2 == 0:
                                ylast = ymm
                            elif hc == HC - 1 and d2 % 2 == 1:
                                dep(ymm, ylast, "y stop after even last")
                    for d2 in range(DC):
                        y_bf = sp.tile([128, TT], bf16, tag="y_bf", bufs=3,
                                       name=f"ybf{b}_{tt}_{d2}")
                        nc.vector.scalar_tensor_tensor(
                            y_bf, y_ps[:, d2, :], b2_sb[:, d2:d2 + 1],
                            g_bc[:, ts(tt, TT)],
                            op0=OP.add, op1=OP.mult)
                        nc.sync.dma_start(rsin[b][d2, :, ts(tt, TT)], y_bf)
                nc.gpsimd.collective_compute(
                    "ReduceScatter", OP.add, replica_groups=rg,
                    ins=[rsin[b].opt()], outs=[rsout[b].opt()])
                for nch in range(2):
                    ob_bf = sp.tile([128, 512], bf16, tag="ob_bf", bufs=1,
                                    name=f"obbf{b}_{nch}")
                    nc.sync.dma_start(ob_bf, rsout[b][:, ts(nch, 512)])
                    ob = sp.tile([128, 512], f32, tag="ob", bufs=1,
                                 name=f"ob{b}_{nch}")
                    nc.vector.tensor_copy(ob, ob_bf)
                    nc.sync.dma_start(out_e[b, :, ts(nch, 512)], ob)

            # software-pipelined emission: attn runs one batch ahead of moe
            if debug_attn:
                emit_attention(0)
            else:
                emit_attention(0)
                for b in range(1, B):
                    emit_attention(b)
                    emit_moe(b - 1)
                emit_moe(B - 1)

    nc.compile()
    return nc


def _prep_inputs(inputs):
    """Host-side shard prep: slice/transpose/cast per core."""
    f32 = np.float32

    def b(x):
        return np.ascontiguousarray(x).astype(bf16np)

    x = inputs["x"].astype(f32)
    w_qkv = inputs["w_qkv"].astype(f32)
    w_out = inputs["w_out"].astype(f32)
    qn_g, kn_g = inputs["qn_g"].astype(f32), inputs["kn_g"].astype(f32)
    gate_w, gate_b = inputs["gate_w"].astype(f32), inputs["gate_b"].astype(f32)
    w1, b1, w2, b2 = (inputs["w1"].astype(f32), inputs["b1"].astype(f32),
                      inputs["w2"].astype(f32), inputs["b2"].astype(f32))

    xT = b(x.transpose(0, 2, 1).reshape(B, DC, 128, S))
    gatewr = b(gate_w.reshape(DC, 128, E))
    ii, jj = np.meshgrid(np.arange(128), np.arange(256), indexing="ij")
    cmaskT2a = np.where(ii <= jj, 0.0, NEG).astype(f32)   # diag on left half
    cmaskT2b = np.where(jj < 128, NEG,
                        np.where(ii <= jj - 128, 0.0, NEG)).astype(f32)
    ii, jj = np.meshgrid(np.arange(128), np.arange(128), indexing="ij")
    mmean = b(np.where(ii // 64 == jj // 64, 1.0 / 64, 0.0).astype(f32))
    onesA = np.ones((2, 128), f32)
    qg = np.tile(qn_g, 2).reshape(128, 1).astype(f32)
    kg = np.tile(kn_g, 2).reshape(128, 1).astype(f32)
    gateb = gate_b.reshape(E, 1).astype(f32)

    in_maps = []
    for c in range(NC):
        h0, h1 = HPC * c, HPC * c + 1
        cs = np.r_[h0 * 64:(h0 + 1) * 64, h1 * 64:(h1 + 1) * 64]
        wqk_c = np.concatenate(
            [w_qkv[:, cs], w_qkv[:, DIM + cs]], axis=1)          # [1024,256]
        wv_c = w_qkv[:, 2 * DIM + cs]                            # [1024,128]
        wout_c = w_out[cs, :]                                    # [128,1024]
        onehot = np.zeros((E, 1), bf16np)
        onehot[c, 0] = 1.0
        in_maps.append({
            "xT": xT,
            "wqk": b(wqk_c.reshape(DC, 128, 256)),
            "wv": b(wv_c.reshape(DC, 128, 128)),
            "wout": b(wout_c.reshape(128, DC, 128).transpose(1, 0, 2)),
            "w1": b(w1[c].reshape(DC, 128, HID)),
            "w2": b(w2[c].reshape(HC, 128, DIM)),
            "gatew": gatewr,
            "b1s": np.ascontiguousarray(b1[c].reshape(HC, 128).T).astype(f32),
            "b2s": np.ascontiguousarray(b2[c].reshape(DC, 128).T).astype(f32),
            "gateb": gateb,
            "qg": qg,
            "kg": kg,
            "cmaskT2a": cmaskT2a,
            "cmaskT2b": cmaskT2b,
            "ones128k": np.ones((128, 1), bf16np),
            "onesB": np.ones((128, 64), f32),
            "mmean": mmean,
            "ones8": np.ones((E, 1), bf16np),
            "onehot": onehot,
            "onesA": onesA,
        })
    return in_maps


def kernel(**inputs):
    from concourse.bass_utils import run_bass_kernel_spmd

    if "nc" not in _cache:
        _cache["nc"] = _build()
    nc = _cache["nc"]
    in_maps = _prep_inputs(inputs)
    res = run_bass_kernel_spmd(nc, in_maps, core_ids=list(range(NC)))
    full = np.empty((B, S, DIM), np.float32)
    for c in range(NC):
        full[:, :, 128 * c:128 * (c + 1)] = (
            res.results[c]["out"].transpose(0, 2, 1))
    return full


# revision 23
# speedup vs baseline: 1.0329x; 1.0141x over previous
"""Trainium2 Bass kernel for a dense-MoE encoder layer (8 NeuronCores).

Sharding:
  - Attention: head-parallel (16 heads / 8 cores = 2 heads per core); the
    output-projection partial sums are AllReduced (bf16) per batch.
  - MoE: expert-parallel (8 experts / 8 cores = 1 expert per core); the
    gate-weighted expert outputs are ReduceScattered (bf16) per batch.

All matmuls run in bf16 with fp32 PSUM accumulation. Softmax skips the
max-subtraction (qk-norm bounds |score*scale| <= 8). Host-side prep does
layout/transpose/dtype conversion only. Emission is software-pipelined:
attn(b+1) is emitted between attn(b) and moe(b) so each batch's AllReduce
overlaps the next batch's attention and the previous batch's MoE.
"""

import numpy as np
import ml_dtypes

B, S, DIM, HEADS, DH = 4, 1024, 1024, 16, 64
E, HID = 8, 4096
NC = 8
HPC = HEADS // NC  # heads per core
SCALE = DH ** -0.5
EPS = 1e-5
NEG = -30000.0
TT = 256  # MoE token tile
NTT = S // TT
DC = DIM // 128  # 8 d-chunks
HC = HID // 128  # 32 hid-chunks

bf16np = ml_dtypes.bfloat16

_cache = {}


def _build(debug_attn=False):
    import concourse.mybir as mybir
    import concourse.tile as tile
    from concourse import bacc
    from concourse.bass import ts

    bf16 = mybir.dt.bfloat16
    f32 = mybir.dt.float32
    AF = mybir.ActivationFunctionType
    OP = mybir.AluOpType

    import bass_rust

    def dep(a, b, why):
        # a must run after b (same-engine ordering for PSUM zero-region tricks)
        bass_rust.add_dep_helper(a.ins, b.ins, reason=why)

    nc = bacc.Bacc(None, target_bir_lowering=False, debug=False, num_devices=NC)
    P = {}
    for name, shape, dt in [
        ("xT", [B, DC, 128, S], bf16),
        ("wqk", [DC, 128, 2 * 128], bf16),
        ("wv", [DC, 128, 128], bf16),
        ("wout", [DC, 128, 128], bf16),
        ("w1", [DC, 128, HID], bf16),
        ("w2", [HC, 128, DIM], bf16),
        ("gatew", [DC, 128, E], bf16),
        ("b1s", [128, HC], f32),
        ("b2s", [128, DC], f32),
        ("gateb", [E, 1], f32),
        ("qg", [128, 1], f32),
        ("kg", [128, 1], f32),
        ("cmaskT2a", [128, 256], f32),
        ("cmaskT2b", [128, 256], f32),
        ("ones128k", [128, 1], bf16),
        ("onesB", [128, 64], bf16),
        ("mmean", [128, 128], bf16),
        ("ones8", [E, 1], bf16),
        ("onehot", [E, 1], bf16),
        ("onesA", [2, 128], bf16),
    ]:
        P[name] = nc.declare_dram_parameter(name, shape, dt, isOutput=False)
    out_e = nc.declare_dram_parameter("out", [B, 128, S], f32, isOutput=True)

    rg = [list(range(NC))]

    with tile.TileContext(nc, num_cores=NC) as tc:
        with (
            tc.tile_pool(name="wp", bufs=1) as wp,
            tc.tile_pool(name="sp", bufs=2) as sp,
            tc.tile_pool(name="pp", bufs=1, space="PSUM") as pp,
            tc.tile_pool(name="dp", bufs=1, space="DRAM") as dp,
        ):
            # ---- resident weights / constants ----
            w1_sb = wp.tile([128, DC, HID], bf16)
            w2_sb = wp.tile([128, HC, DIM], bf16)
            wqk_sb = wp.tile([128, DC, 256], bf16)
            wv_sb = wp.tile([128, DC, 128], bf16)
            wout_sb = wp.tile([128, DC, 128], bf16)
            gatew_sb = wp.tile([128, DC, E], bf16)
            for d in range(DC):
                nc.sync.dma_start(w1_sb[:, d, :], P["w1"][d])
                nc.sync.dma_start(wqk_sb[:, d, :], P["wqk"][d])
                nc.sync.dma_start(wv_sb[:, d, :], P["wv"][d])
                nc.sync.dma_start(wout_sb[:, d, :], P["wout"][d])
                nc.sync.dma_start(gatew_sb[:, d, :], P["gatew"][d])
            for h in range(HC):
                nc.sync.dma_start(w2_sb[:, h, :], P["w2"][h])
            b1_sb = wp.tile([128, HC], f32)
            b2_sb = wp.tile([128, DC], f32)
            gateb_sb = wp.tile([E, 1], f32)
            qg_sb = wp.tile([128, 1], f32)
            kg_sb = wp.tile([128, 1], f32)
            cm2a_sb = wp.tile([128, 256], f32)
            cm2b_sb = wp.tile([128, 256], f32)
            ones128k_sb = wp.tile([128, 1], bf16)
            onesB_sb = wp.tile([128, 64], bf16)
            mmean_sb = wp.tile([128, 128], bf16)
            ones8_sb = wp.tile([E, 1], bf16)
            onehot_sb = wp.tile([E, 1], bf16)
            onesA_sb = wp.tile([2, 128], bf16)
            eps_sb = wp.tile([128, 1], f32)
            for nm, t in [
                ("b1s", b1_sb), ("b2s", b2_sb), ("gateb", gateb_sb),
                ("qg", qg_sb), ("kg", kg_sb),
                ("cmaskT2a", cm2a_sb), ("cmaskT2b", cm2b_sb),
                ("ones128k", ones128k_sb), ("onesB", onesB_sb),
                ("mmean", mmean_sb),
                ("ones8", ones8_sb), ("onehot", onehot_sb), ("onesA", onesA_sb),
            ]:
                nc.sync.dma_start(t, P[nm][:, :])
            nc.vector.memset(eps_sb, EPS)

            arin, arout, rsin, rsout = [], [], [], []
            for b in range(B):
                arin.append([dp.tile([DC, 128, 512], bf16,
                                     name=f"arin{b}_{x}") for x in range(2)])
                arout.append([dp.tile([DC, 128, 512], bf16,
                                      name=f"arout{b}_{x}",
                                      addr_space="Shared") for x in range(2)])
                rsin.append(dp.tile([DC, 128, S], bf16, name=f"rsin{b}"))
                rsout.append(dp.tile([128, S], bf16, name=f"rsout{b}"))

            # PSUM tags (8 banks total):
            #   y: [128,8,256] f32 = 4 banks, bufs=1
            #   h: [128,256] f32 = 1 bank, bufs=2 (double-buffered h GEMM)
            #   ws: [128,512] f32 = 1 bank, bufs=1 (all transient matmul outs)
            #   pv: [128,512] f32 = 1 bank, bufs=1 (PV accum + softmax denom)

            def emit_attention(b):
                # ---- qkv projection (feature-major q,k) + qk-norm ----
                qkT = sp.tile([128, 2, S], bf16, tag="qkT", bufs=1,
                              name=f"qkT{b}")
                for nch in range(2):
                    for m in range(2):
                        qk_ps = pp.tile([128, 512], f32, tag="ws", bufs=1,
                                        name=f"qkps{b}_{nch}_{m}")
                        for d in range(DC):
                            xt = sp.tile([128, 512], bf16, tag="xt", bufs=2,
                                         name=f"xt{b}_{nch}_{m}_{d}")
                            nc.sync.dma_start(xt,
                                              P["xT"][b, d, :, ts(nch, 512)])
                            nc.tensor.matmul(
                                qk_ps, wqk_sb[:, d, ts(m, 128)], xt,
                                start=(d == 0), stop=(d == DC - 1))
                        nc.scalar.copy(qkT[:, m, ts(nch, 512)], qk_ps)
                    for m in range(2):
                        qraw = qkT[:, m, ts(nch, 512)]
                        mu_ps = pp.tile([128, 512], f32, tag="ws", bufs=1,
                                        name=f"mups{b}_{nch}_{m}")
                        nc.tensor.matmul(mu_ps, mmean_sb, qraw,
                                         start=True, stop=True)
                        sub = sp.tile([128, 512], bf16, tag="sub",
                                      name=f"sub{b}_{nch}_{m}")
                        nc.vector.tensor_tensor(sub, qraw, mu_ps, OP.subtract)
                        sq = sp.tile([128, 512], bf16, tag="sq",
                                     name=f"sq{b}_{nch}_{m}")
                        nc.vector.tensor_mul(sq, sub, sub)
                        var_ps = pp.tile([128, 512], f32, tag="ws", bufs=1,
                                         name=f"varps{b}_{nch}_{m}")
                        nc.tensor.matmul(var_ps, mmean_sb, sq,
                                         start=True, stop=True)
                        rstd = sp.tile([128, 512], f32, tag="rstd",
                                       name=f"rstd{b}_{nch}_{m}")
                        nc.scalar.activation(rstd, var_ps, AF.Sqrt,
                                             bias=eps_sb, scale=1.0)
                        nc.vector.reciprocal(rstd, rstd)
                        g_ap = qg_sb if m == 0 else kg_sb
                        nc.vector.scalar_tensor_tensor(
                            qkT[:, m, ts(nch, 512)], sub, g_ap, rstd,
                            op0=OP.mult, op1=OP.mult)
                # ---- v (token-major) ----
                v_sb = sp.tile([128, DC, 128], bf16, tag="v_sb",
                               name=f"vsb{b}")
                for vg in range(2):  # two groups of 4 token-chunks
                    vq = pp.tile([128, 4, 128], f32, tag="ws", bufs=1,
                                 name=f"vq{b}_{vg}")
                    vfirst, vlasts = None, []
                    for tq in range(4):
                        tcn = vg * 4 + tq
                        for d in range(DC):
                            xtv = sp.tile([128, 128], bf16, tag="xtv", bufs=4,
                                          name=f"xtv{b}_{tcn}_{d}")
                            nc.sync.dma_start(
                                xtv, P["xT"][b, d, :, ts(tcn, 128)])
                            mm = nc.tensor.matmul(
                                vq[:, tq, :], xtv, wv_sb[:, d, :],
                                start=(tq == 0 and d == 0),
                                stop=(tq == 3 and d == DC - 1),
                                skip_group_check=(tq != 0))
                            if tq == 0 and d == 0:
                                vfirst = mm
                            elif d == 0:
                                dep(mm, vfirst, "v zero-region after start")
                            if d == DC - 1 and tq < 3:
                                vlasts.append(mm)
                            if tq == 3 and d == DC - 1:
                                for vl in vlasts:
                                    dep(mm, vl, "v stop after all groups")
                        nc.scalar.copy(v_sb[:, tcn, :], vq[:, tq, :])
                # ---- causal attention, 2 heads ----
                oTh = sp.tile([128, S], bf16, tag="oTh", bufs=1,
                              name=f"oTh{b}")
                for qp in range(4):  # 256-query pairs, both heads share pvt
                    pvt = pp.tile([128, 512], f32, tag="pv", bufs=1,
                                  name=f"pvt{b}_{qp}")
                    nkc = 2 * qp + 2
                    for h in range(HPC):
                        hsl = slice(64 * h, 64 * h + 64)
                        dsl = slice(64 * h, 64 * h + 1)
                        pvfirst, pvden = None, None
                        for kc in range(nkc):
                            scT = pp.tile([128, 256], f32, tag="ws", bufs=1,
                                          name=f"scT{b}_{qp}_{h}_{kc}")
                            nc.tensor.matmul(
                                scT,
                                qkT[hsl, 1, ts(kc, 128)],
                                qkT[hsl, 0, ts(qp, 256)],
                                start=True, stop=True)
                            if kc == nkc - 2:
                                nc.vector.tensor_add(scT, scT, cm2a_sb)
                            elif kc == nkc - 1:
                                nc.vector.tensor_add(scT, scT, cm2b_sb)
                            exk = sp.tile([128, 256], bf16, tag="exk", bufs=4,
                                          name=f"exk{b}_{qp}_{h}_{kc}")
                            nc.scalar.activation(exk, scT, AF.Exp, scale=SCALE)
                            pvmm = nc.tensor.matmul(
                                pvt[hsl, 0:256], v_sb[:, kc, hsl], exk,
                                start=(kc == 0), stop=(kc == nkc - 1))
                            if kc == 0:
                                pvfirst = pvmm
                            if kc == nkc - 1 and pvden is not None:
                                dep(pvmm, pvden, "pv stop after last den")
                            dmm = nc.tensor.matmul(
                                pvt[dsl, 256:512], ones128k_sb, exk,
                                start=False, stop=False, skip_group_check=True)
                            if kc == 0:
                                dep(dmm, pvfirst, "den zero after pv start")
                            pvden = dmm
                    for h in range(HPC):
                        hsl = slice(64 * h, 64 * h + 64)
                        dsl = slice(64 * h, 64 * h + 1)
                        rec_sb = sp.tile([128, 256], f32, tag="rec_sb",
                                         bufs=2, name=f"rcs{b}_{qp}_{h}")
                        nc.vector.reciprocal(rec_sb[dsl, :],
                                             pvt[dsl, 256:512])
                        rec_bf = sp.tile([128, 256], bf16, tag="rec_bf",
                                         bufs=2, name=f"rcb{b}_{qp}_{h}")
                        nc.vector.tensor_copy(rec_bf[dsl, :], rec_sb[dsl, :])
                        rb = pp.tile([128, 256], f32, tag="ws", bufs=1,
                                     name=f"rb{b}_{qp}_{h}")
                        nc.tensor.matmul(rb[hsl, :], onesB_sb[dsl, :],
                                         rec_bf[dsl, :],
                                         start=True, stop=True)
                        rb_sb = sp.tile([128, 256], f32, tag="rb_sb",
                                        bufs=2, name=f"rbs{b}_{qp}_{h}")
                        nc.scalar.copy(rb_sb[hsl, :], rb[hsl, :])
                        nc.vector.tensor_mul(oTh[hsl, ts(qp, 256)],
                                             pvt[hsl, 0:256], rb_sb[hsl, :])
                # ---- out-projection partial -> arin -> AllReduce ----
                for nch in range(2):
                    for mc in range(DC):
                        wo_ps = pp.tile([128, 512], f32, tag="ws", bufs=1,
                                        name=f"wops{b}_{mc}_{nch}")
                        nc.tensor.matmul(wo_ps, wout_sb[:, mc, :],
                                         oTh[:, ts(nch, 512)],
                                         start=True, stop=True)
                        wo_bf = sp.tile([128, 512], bf16, tag="wo_bf", bufs=2,
                                        name=f"wobf{b}_{mc}_{nch}")
                        nc.vector.tensor_copy(wo_bf, wo_ps)
                        nc.sync.dma_start(arin[b][nch][mc, :, :], wo_bf)
                    nc.gpsimd.collective_compute(
                        "AllReduce", OP.add, replica_groups=rg,
                        ins=[arin[b][nch].opt()],
                        outs=[arout[b][nch].opt()])

            def emit_moe(b):
                oT = sp.tile([128, DC, S], bf16, tag="oT", bufs=1,
                             name=f"oT{b}")
                for x in range(2):
                    for d in range(DC):
                        nc.sync.dma_start(oT[:, d, ts(x, 512)],
                                          arout[b][x][d])
                pass
                # ---- gates ----
                g_bc = sp.tile([128, S], bf16, tag="g_bc", bufs=1,
                               name=f"gbc{b}")
                for nch in range(2):
                    lg = pp.tile([E, 512], f32, tag="ws", bufs=1,
                                 name=f"lg{b}_{nch}")
                    for d in range(DC):
                        nc.tensor.matmul(lg, gatew_sb[:, d, :],
                                         oT[:, d, ts(nch, 512)],
                                         start=(d == 0), stop=(d == DC - 1))
                    expT = sp.tile([E, 512], bf16, tag="expT",
                                   name=f"expT{b}_{nch}")
                    nc.scalar.activation(expT, lg, AF.Exp, bias=gateb_sb,
                                         scale=1.0)
                    den_ps = pp.tile([1, 512], f32, tag="ws", bufs=1,
                                     name=f"denps{b}_{nch}")
                    nc.tensor.matmul(den_ps, ones8_sb, expT,
                                     start=True, stop=True)
                    sel_ps = pp.tile([1, 512], f32, tag="ws", bufs=1,
                                     name=f"selps{b}_{nch}")
                    nc.tensor.matmul(sel_ps, onehot_sb, expT,
                                     start=True, stop=True)
                    den_sb = sp.tile([1, 512], bf16, tag="den_sb",
                                     name=f"densb{b}_{nch}")
                    nc.scalar.copy(den_sb, den_ps)
                    sel_sb = sp.tile([1, 512], bf16, tag="sel_sb",
                                     name=f"selsb{b}_{nch}")
                    nc.scalar.copy(sel_sb, sel_ps)
                    # broadcast denominator and selected-expert rows to 128
                    # partitions via K=1 matmuls, then g = sel * (1/den)
                    gbd = pp.tile([128, 512], f32, tag="ws", bufs=1,
                                  name=f"gbd{b}_{nch}")
                    nc.tensor.matmul(gbd, onesA_sb[0:1, :], den_sb,
                                     start=True, stop=True)
                    gbs = pp.tile([128, 512], f32, tag="ws", bufs=1,
                                  name=f"gbs{b}_{nch}")
                    nc.tensor.matmul(gbs, onesA_sb[0:1, :], sel_sb,
                                     start=True, stop=True)
                    recd = sp.tile([128, 512], f32, tag="recd", bufs=1,
                                   name=f"recd{b}_{nch}")
                    nc.vector.reciprocal(recd, gbd)
                    nc.vector.tensor_mul(g_bc[:, ts(nch, 512)], gbs, recd)
                # ---- FFN expert ----
                for tt in range(NTT):
                    y_ps = pp.tile([128, DC, TT], f32, tag="y", bufs=1,
                                   name=f"yps{b}_{tt}")

                    def emit_h(hc, tt=tt):
                        hp = pp.tile([128, TT], f32, tag="h", bufs=2,
                                     name=f"hps{b}_{tt}_{hc}")
                        for d in range(DC):
                            nc.tensor.matmul(
                                hp,
                                w1_sb[:, d, ts(hc, 128)],
                                oT[:, d, ts(tt, TT)],
                                start=(d == 0), stop=(d == DC - 1))
                        return hp

                    hp = emit_h(0)
                    for hc in range(HC):
                        hT = sp.tile([128, TT], bf16, tag="hT", bufs=3,
                                     name=f"hT{b}_{tt}_{hc}")
                        nc.scalar.activation(hT, hp, AF.Gelu_apprx_tanh,
                                             bias=b1_sb[:, hc:hc + 1],
                                             scale=1.0)
                        if hc + 1 < HC:
                            hp = emit_h(hc + 1)
                        for d2 in range(DC):
                            ymm = nc.tensor.matmul(
                                y_ps[:, d2, :],
                                w2_sb[:, hc, ts(d2, 128)],
                                hT,
                                start=(hc == 0 and d2 % 2 == 0),
                                stop=(hc == HC - 1 and d2 % 2 == 1),
                                skip_group_check=(d2 % 2 == 1))
                            if hc == 0 and d2 % 2 == 0:
                                ylast = ymm
                            elif hc == 0 and d2 % 2 == 1:
                                dep(ymm, ylast, "y odd zero after even start")
                            if hc == HC - 1 and d2 % 2 == 0:
                                ylast = ymm
                            elif hc == HC - 1 and d2 % 2 == 1:
                                dep(ymm, ylast, "y stop after even last")
                    for d2 in range(DC):
                        y_bf = sp.tile([128, TT], bf16, tag="y_bf", bufs=3,
                                       name=f"ybf{b}_{tt}_{d2}")
                        nc.vector.scalar_tensor_tensor(
                            y_bf, y_ps[:, d2, :], b2_sb[:, d2:d2 + 1],
                            g_bc[:, ts(tt, TT)],
                            op0=OP.add, op1=OP.mult)
                        nc.sync.dma_start(rsin[b][d2, :, ts(tt, TT)], y_bf)
                nc.gpsimd.collective_compute(
                    "ReduceScatter", OP.add, replica_groups=rg,
                    ins=[rsin[b].opt()], outs=[rsout[b].opt()])
                for nch in range(2):
                    ob_bf = sp.tile([128, 512], bf16, tag="ob_bf", bufs=1,
                                    name=f"obbf{b}_{nch}")
                    nc.sync.dma_start(ob_bf, rsout[b][:, ts(nch, 512)])
                    ob = sp.tile([128, 512], f32, tag="ob", bufs=1,
                                 name=f"ob{b}_{nch}")
                    nc.vector.tensor_copy(ob, ob_bf)
                    nc.sync.dma_start(out_e[b, :, ts(nch, 512)], ob)

            # software-pipelined emission: attn runs one batch ahead of moe
            if debug_attn:
                emit_attention(0)
            else:
                emit_attention(0)
                for b in range(1, B):
                    emit_attention(b)
                    emit_moe(b - 1)
                emit_moe(B - 1)

    nc.compile()
    return nc


def _prep_inputs(inputs):
    """Host-side shard prep: slice/transpose/cast per core."""
    f32 = np.float32

    def b(x):
        return np.ascontiguousarray(x).astype(bf16np)

    x = inputs["x"].astype(f32)
    w_qkv = inputs["w_qkv"].astype(f32)
    w_out = inputs["w_out"].astype(f32)
    qn_g, kn_g = inputs["qn_g"].astype(f32), inputs["kn_g"].astype(f32)
    gate_w, gate_b = inputs["gate_w"].astype(f32), inputs["gate_b"].astype(f32)
    w1, b1, w2, b2 = (inputs["w1"].astype(f32), inputs["b1"].astype(f32),
                      inputs["w2"].astype(f32), inputs["b2"].astype(f32))

    xT = b(x.transpose(0, 2, 1).reshape(B, DC, 128, S))
    gatewr = b(gate_w.reshape(DC, 128, E))
    ii, jj = np.meshgrid(np.arange(128), np.arange(256), indexing="ij")
    cmaskT2a = np.where(ii <= jj, 0.0, NEG).astype(f32)   # diag on left half
    cmaskT2b = np.where(jj < 128, NEG,
                        np.where(ii <= jj - 128, 0.0, NEG)).astype(f32)
    ii, jj = np.meshgrid(np.arange(128), np.arange(128), indexing="ij")
    mmean = b(np.where(ii // 64 == jj // 64, 1.0 / 64, 0.0).astype(f32))
    onesA = np.ones((2, 128), bf16np)
    qg = np.tile(qn_g, 2).reshape(128, 1).astype(f32)
    kg = np.tile(kn_g, 2).reshape(128, 1).astype(f32)
    gateb = gate_b.reshape(E, 1).astype(f32)

    in_maps = []
    for c in range(NC):
        h0, h1 = HPC * c, HPC * c + 1
        cs = np.r_[h0 * 64:(h0 + 1) * 64, h1 * 64:(h1 + 1) * 64]
        wqk_c = np.concatenate(
            [w_qkv[:, cs], w_qkv[:, DIM + cs]], axis=1)          # [1024,256]
        wv_c = w_qkv[:, 2 * DIM + cs]                            # [1024,128]
        wout_c = w_out[cs, :]                                    # [128,1024]
        onehot = np.zeros((E, 1), bf16np)
        onehot[c, 0] = 1.0
        in_maps.append({
            "xT": xT,
            "wqk": b(wqk_c.reshape(DC, 128, 256)),
            "wv": b(wv_c.reshape(DC, 128, 128)),
            "wout": b(wout_c.reshape(128, DC, 128).transpose(1, 0, 2)),
            "w1": b(w1[c].reshape(DC, 128, HID)),
            "w2": b(w2[c].reshape(HC, 128, DIM)),
            "gatew": gatewr,
            "b1s": np.ascontiguousarray(b1[c].reshape(HC, 128).T).astype(f32),
            "b2s": np.ascontiguousarray(b2[c].reshape(DC, 128).T).astype(f32),
            "gateb": gateb,
            "qg": qg,
            "kg": kg,
            "cmaskT2a": cmaskT2a,
            "cmaskT2b": cmaskT2b,
            "ones128k": np.ones((128, 1), bf16np),
            "onesB": np.ones((128, 64), bf16np),
            "mmean": mmean,
            "ones8": np.ones((E, 1), bf16np),
            "onehot": onehot,
            "onesA": onesA,
        })
    return in_maps


def kernel(**inputs):
    from concourse.bass_utils import run_bass_kernel_spmd

    if "nc" not in _cache:
        _cache["nc"] = _build()
    nc = _cache["nc"]
    in_maps = _prep_inputs(inputs)
    res = run_bass_kernel_spmd(nc, in_maps, core_ids=list(range(NC)))
    full = np.empty((B, S, DIM), np.float32)
    for c in range(NC):
        full[:, :, 128 * c:128 * (c + 1)] = (
            res.results[c]["out"].transpose(0, 2, 1))
    return full


# revision 24
# speedup vs baseline: 1.0760x; 1.0417x over previous
"""Trainium2 Bass kernel for a dense-MoE encoder layer (8 NeuronCores).

Sharding:
  - Attention: head-parallel (16 heads / 8 cores = 2 heads per core); the
    output-projection partial sums are AllReduced (bf16) per batch.
  - MoE: expert-parallel (8 experts / 8 cores = 1 expert per core); the
    gate-weighted expert outputs are ReduceScattered (bf16) per batch.

All matmuls run in bf16 with fp32 PSUM accumulation. Softmax skips the
max-subtraction (qk-norm bounds |score*scale| <= 8). Host-side prep does
layout/transpose/dtype conversion only. Emission is software-pipelined:
attn(b+1) is emitted between attn(b) and moe(b) so each batch's AllReduce
overlaps the next batch's attention and the previous batch's MoE.
"""

import numpy as np
import ml_dtypes

B, S, DIM, HEADS, DH = 4, 1024, 1024, 16, 64
E, HID = 8, 4096
NC = 8
HPC = HEADS // NC  # heads per core
SCALE = DH ** -0.5
EPS = 1e-5
NEG = -30000.0
TT = 256  # MoE token tile
NTT = S // TT
DC = DIM // 128  # 8 d-chunks
HC = HID // 128  # 32 hid-chunks

bf16np = ml_dtypes.bfloat16

_cache = {}


def _build(debug_attn=False):
    import concourse.mybir as mybir
    import concourse.tile as tile
    from concourse import bacc
    from concourse.bass import ts

    bf16 = mybir.dt.bfloat16
    f32 = mybir.dt.float32
    AF = mybir.ActivationFunctionType
    OP = mybir.AluOpType

    import bass_rust

    def dep(a, b, why):
        # a must run after b (same-engine ordering for PSUM zero-region tricks)
        bass_rust.add_dep_helper(a.ins, b.ins, reason=why)

    nc = bacc.Bacc(None, target_bir_lowering=False, debug=False, num_devices=NC)
    P = {}
    for name, shape, dt in [
        ("xT", [B, DC, 128, S], bf16),
        ("wqk", [DC, 128, 2 * 128], bf16),
        ("wv", [DC, 128, 128], bf16),
        ("wout", [DC, 128, 128], bf16),
        ("w1", [DC, 128, HID], bf16),
        ("w2", [HC, 128, DIM], bf16),
        ("gatew", [DC, 128, E], bf16),
        ("b1s", [128, HC], f32),
        ("b2s", [128, DC], f32),
        ("gateb", [E, 1], f32),
        ("qg", [128, 1], f32),
        ("kg", [128, 1], f32),
        ("cmaskT2a", [128, 256], f32),
        ("cmaskT2b", [128, 256], f32),
        ("ones128k", [128, 1], bf16),
        ("onesB", [128, 64], bf16),
        ("mmean", [128, 128], bf16),
        ("ones8", [E, 1], bf16),
        ("onehot", [E, 1], bf16),
        ("onesA", [2, 128], bf16),
    ]:
        P[name] = nc.declare_dram_parameter(name, shape, dt, isOutput=False)
    out_e = nc.declare_dram_parameter("out", [B, 128, S], f32, isOutput=True)

    rg = [list(range(NC))]

    with tile.TileContext(nc, num_cores=NC) as tc:
        with (
            tc.tile_pool(name="wp", bufs=1) as wp,
            tc.tile_pool(name="sp", bufs=2) as sp,
            tc.tile_pool(name="pp", bufs=1, space="PSUM") as pp,
            tc.tile_pool(name="dp", bufs=1, space="DRAM") as dp,
        ):
            # ---- resident weights / constants ----
            w1_sb = wp.tile([128, DC, HID], bf16)
            w2_sb = wp.tile([128, HC, DIM], bf16)
            wqk_sb = wp.tile([128, DC, 256], bf16)
            wv_sb = wp.tile([128, DC, 128], bf16)
            wout_sb = wp.tile([128, DC, 128], bf16)
            gatew_sb = wp.tile([128, DC, E], bf16)
            for d in range(DC):
                nc.sync.dma_start(wqk_sb[:, d, :], P["wqk"][d])
                nc.sync.dma_start(wv_sb[:, d, :], P["wv"][d])
                nc.sync.dma_start(wout_sb[:, d, :], P["wout"][d])

            def load_moe_weights():
                for d in range(DC):
                    nc.sync.dma_start(w1_sb[:, d, :], P["w1"][d])
                    nc.sync.dma_start(gatew_sb[:, d, :], P["gatew"][d])
                for h in range(HC):
                    nc.sync.dma_start(w2_sb[:, h, :], P["w2"][h])
            b1_sb = wp.tile([128, HC], f32)
            b2_sb = wp.tile([128, DC], f32)
            gateb_sb = wp.tile([E, 1], f32)
            qg_sb = wp.tile([128, 1], f32)
            kg_sb = wp.tile([128, 1], f32)
            cm2a_sb = wp.tile([128, 256], f32)
            cm2b_sb = wp.tile([128, 256], f32)
            ones128k_sb = wp.tile([128, 1], bf16)
            onesB_sb = wp.tile([128, 64], bf16)
            mmean_sb = wp.tile([128, 128], bf16)
            ones8_sb = wp.tile([E, 1], bf16)
            onehot_sb = wp.tile([E, 1], bf16)
            onesA_sb = wp.tile([2, 128], bf16)
            eps_sb = wp.tile([128, 1], f32)
            for nm, t in [
                ("b1s", b1_sb), ("b2s", b2_sb), ("gateb", gateb_sb),
                ("qg", qg_sb), ("kg", kg_sb),
                ("cmaskT2a", cm2a_sb), ("cmaskT2b", cm2b_sb),
                ("ones128k", ones128k_sb), ("onesB", onesB_sb),
                ("mmean", mmean_sb),
                ("ones8", ones8_sb), ("onehot", onehot_sb), ("onesA", onesA_sb),
            ]:
                nc.sync.dma_start(t, P[nm][:, :])
            nc.vector.memset(eps_sb, EPS)

            arin, arout, rsin, rsout = [], [], [], []
            for b in range(B):
                arin.append([dp.tile([DC, 128, 512], bf16,
                                     name=f"arin{b}_{x}") for x in range(2)])
                arout.append([dp.tile([DC, 128, 512], bf16,
                                      name=f"arout{b}_{x}",
                                      addr_space="Shared") for x in range(2)])
                rsin.append(dp.tile([DC, 128, S], bf16, name=f"rsin{b}"))
                rsout.append(dp.tile([128, S], bf16, name=f"rsout{b}"))

            # PSUM tags (8 banks total):
            #   y: [128,8,256] f32 = 4 banks, bufs=1
            #   h: [128,256] f32 = 1 bank, bufs=2 (double-buffered h GEMM)
            #   ws: [128,512] f32 = 1 bank, bufs=1 (all transient matmul outs)
            #   pv: [128,512] f32 = 1 bank, bufs=1 (PV accum + softmax denom)

            def emit_attention(b):
                # ---- qkv projection (feature-major q,k) + qk-norm ----
                qkT = sp.tile([128, 2, S], bf16, tag="qkT", bufs=1,
                              name=f"qkT{b}")
                for nch in range(2):
                    for m in range(2):
                        qk_ps = pp.tile([128, 512], f32, tag="ws", bufs=1,
                                        name=f"qkps{b}_{nch}_{m}")
                        for d in range(DC):
                            xt = sp.tile([128, 512], bf16, tag="xt", bufs=2,
                                         name=f"xt{b}_{nch}_{m}_{d}")
                            nc.sync.dma_start(xt,
                                              P["xT"][b, d, :, ts(nch, 512)])
                            nc.tensor.matmul(
                                qk_ps, wqk_sb[:, d, ts(m, 128)], xt,
                                start=(d == 0), stop=(d == DC - 1))
                        nc.scalar.copy(qkT[:, m, ts(nch, 512)], qk_ps)
                    for m in range(2):
                        qraw = qkT[:, m, ts(nch, 512)]
                        mu_ps = pp.tile([128, 512], f32, tag="ws", bufs=1,
                                        name=f"mups{b}_{nch}_{m}")
                        nc.tensor.matmul(mu_ps, mmean_sb, qraw,
                                         start=True, stop=True)
                        sub = sp.tile([128, 512], bf16, tag="sub",
                                      name=f"sub{b}_{nch}_{m}")
                        nc.vector.tensor_tensor(sub, qraw, mu_ps, OP.subtract)
                        sq = sp.tile([128, 512], bf16, tag="sq",
                                     name=f"sq{b}_{nch}_{m}")
                        nc.vector.tensor_mul(sq, sub, sub)
                        var_ps = pp.tile([128, 512], f32, tag="ws", bufs=1,
                                         name=f"varps{b}_{nch}_{m}")
                        nc.tensor.matmul(var_ps, mmean_sb, sq,
                                         start=True, stop=True)
                        rstd = sp.tile([128, 512], f32, tag="rstd",
                                       name=f"rstd{b}_{nch}_{m}")
                        nc.scalar.activation(rstd, var_ps, AF.Sqrt,
                                             bias=eps_sb, scale=1.0)
                        nc.vector.reciprocal(rstd, rstd)
                        g_ap = qg_sb if m == 0 else kg_sb
                        nc.vector.scalar_tensor_tensor(
                            qkT[:, m, ts(nch, 512)], sub, g_ap, rstd,
                            op0=OP.mult, op1=OP.mult)
                # ---- v (token-major) ----
                v_sb = sp.tile([128, DC, 128], bf16, tag="v_sb",
                               name=f"vsb{b}")
                for vg in range(2):  # two groups of 4 token-chunks
                    vq = pp.tile([128, 4, 128], f32, tag="ws", bufs=1,
                                 name=f"vq{b}_{vg}")
                    vfirst, vlasts = None, []
                    for tq in range(4):
                        tcn = vg * 4 + tq
                        for d in range(DC):
                            xtv = sp.tile([128, 128], bf16, tag="xtv", bufs=4,
                                          name=f"xtv{b}_{tcn}_{d}")
                            nc.sync.dma_start(
                                xtv, P["xT"][b, d, :, ts(tcn, 128)])
                            mm = nc.tensor.matmul(
                                vq[:, tq, :], xtv, wv_sb[:, d, :],
                                start=(tq == 0 and d == 0),
                                stop=(tq == 3 and d == DC - 1),
                                skip_group_check=(tq != 0))
                            if tq == 0 and d == 0:
                                vfirst = mm
                            elif d == 0:
                                dep(mm, vfirst, "v zero-region after start")
                            if d == DC - 1 and tq < 3:
                                vlasts.append(mm)
                            if tq == 3 and d == DC - 1:
                                for vl in vlasts:
                                    dep(mm, vl, "v stop after all groups")
                        nc.scalar.copy(v_sb[:, tcn, :], vq[:, tq, :])
                # ---- causal attention, 2 heads ----
                oTh = sp.tile([128, S], bf16, tag="oTh", bufs=1,
                              name=f"oTh{b}")
                for qp in range(4):  # 256-query pairs, both heads share pvt
                    pvt = pp.tile([128, 512], f32, tag="pv", bufs=1,
                                  name=f"pvt{b}_{qp}")
                    nkc = 2 * qp + 2
                    for h in range(HPC):
                        hsl = slice(64 * h, 64 * h + 64)
                        dsl = slice(64 * h, 64 * h + 1)
                        pvfirst, pvden = None, None
                        for kc in range(nkc):
                            scT = pp.tile([128, 256], f32, tag="ws", bufs=1,
                                          name=f"scT{b}_{qp}_{h}_{kc}")
                            nc.tensor.matmul(
                                scT,
                                qkT[hsl, 1, ts(kc, 128)],
                                qkT[hsl, 0, ts(qp, 256)],
                                start=True, stop=True)
                            if kc == nkc - 2:
                                nc.vector.tensor_add(scT, scT, cm2a_sb)
                            elif kc == nkc - 1:
                                nc.vector.tensor_add(scT, scT, cm2b_sb)
                            exk = sp.tile([128, 256], bf16, tag="exk", bufs=4,
                                          name=f"exk{b}_{qp}_{h}_{kc}")
                            nc.scalar.activation(exk, scT, AF.Exp, scale=SCALE)
                            pvmm = nc.tensor.matmul(
                                pvt[hsl, 0:256], v_sb[:, kc, hsl], exk,
                                start=(kc == 0), stop=(kc == nkc - 1))
                            if kc == 0:
                                pvfirst = pvmm
                            if kc == nkc - 1 and pvden is not None:
                                dep(pvmm, pvden, "pv stop after last den")
                            dmm = nc.tensor.matmul(
                                pvt[dsl, 256:512], ones128k_sb, exk,
                                start=False, stop=False, skip_group_check=True)
                            if kc == 0:
                                dep(dmm, pvfirst, "den zero after pv start")
                            pvden = dmm
                    for h in range(HPC):
                        hsl = slice(64 * h, 64 * h + 64)
                        dsl = slice(64 * h, 64 * h + 1)
                        rec_sb = sp.tile([128, 256], f32, tag="rec_sb",
                                         bufs=2, name=f"rcs{b}_{qp}_{h}")
                        nc.vector.reciprocal(rec_sb[dsl, :],
                                             pvt[dsl, 256:512])
                        rec_bf = sp.tile([128, 256], bf16, tag="rec_bf",
                                         bufs=2, name=f"rcb{b}_{qp}_{h}")
                        nc.vector.tensor_copy(rec_bf[dsl, :], rec_sb[dsl, :])
                        rb = pp.tile([128, 256], f32, tag="ws", bufs=1,
                                     name=f"rb{b}_{qp}_{h}")
                        nc.tensor.matmul(rb[hsl, :], onesB_sb[dsl, :],
                                         rec_bf[dsl, :],
                                         start=True, stop=True)
                        rb_sb = sp.tile([128, 256], f32, tag="rb_sb",
                                        bufs=2, name=f"rbs{b}_{qp}_{h}")
                        nc.scalar.copy(rb_sb[hsl, :], rb[hsl, :])
                        nc.vector.tensor_mul(oTh[hsl, ts(qp, 256)],
                                             pvt[hsl, 0:256], rb_sb[hsl, :])
                    if qp % 2 == 1:
                        nch = qp // 2
                        for mc in range(DC):
                            wo_ps = pp.tile([128, 512], f32, tag="ws", bufs=1,
                                            name=f"wops{b}_{mc}_{nch}")
                            nc.tensor.matmul(wo_ps, wout_sb[:, mc, :],
                                             oTh[:, ts(nch, 512)],
                                             start=True, stop=True)
                            wo_bf = sp.tile([128, 512], bf16, tag="wo_bf",
                                            bufs=2, name=f"wobf{b}_{mc}_{nch}")
                            nc.vector.tensor_copy(wo_bf, wo_ps)
                            nc.sync.dma_start(arin[b][nch][mc, :, :], wo_bf)
                        nc.gpsimd.collective_compute(
                            "AllReduce", OP.add, replica_groups=rg,
                            ins=[arin[b][nch].opt()],
                            outs=[arout[b][nch].opt()])


            def emit_moe(b):
                oT = sp.tile([128, DC, S], bf16, tag="oT", bufs=1,
                             name=f"oT{b}")
                for x in range(2):
                    for d in range(DC):
                        nc.sync.dma_start(oT[:, d, ts(x, 512)],
                                          arout[b][x][d])
                pass
                # ---- gates ----
                g_bc = sp.tile([128, S], bf16, tag="g_bc", bufs=1,
                               name=f"gbc{b}")
                for nch in range(2):
                    lg = pp.tile([E, 512], f32, tag="ws", bufs=1,
                                 name=f"lg{b}_{nch}")
                    for d in range(DC):
                        nc.tensor.matmul(lg, gatew_sb[:, d, :],
                                         oT[:, d, ts(nch, 512)],
                                         start=(d == 0), stop=(d == DC - 1))
                    expT = sp.tile([E, 512], bf16, tag="expT",
                                   name=f"expT{b}_{nch}")
                    nc.scalar.activation(expT, lg, AF.Exp, bias=gateb_sb,
                                         scale=1.0)
                    den_ps = pp.tile([1, 512], f32, tag="ws", bufs=1,
                                     name=f"denps{b}_{nch}")
                    nc.tensor.matmul(den_ps, ones8_sb, expT,
                                     start=True, stop=True)
                    sel_ps = pp.tile([1, 512], f32, tag="ws", bufs=1,
                                     name=f"selps{b}_{nch}")
                    nc.tensor.matmul(sel_ps, onehot_sb, expT,
                                     start=True, stop=True)
                    den_sb = sp.tile([1, 512], bf16, tag="den_sb",
                                     name=f"densb{b}_{nch}")
                    nc.scalar.copy(den_sb, den_ps)
                    sel_sb = sp.tile([1, 512], bf16, tag="sel_sb",
                                     name=f"selsb{b}_{nch}")
                    nc.scalar.copy(sel_sb, sel_ps)
                    # broadcast denominator and selected-expert rows to 128
                    # partitions via K=1 matmuls, then g = sel * (1/den)
                    gbd = pp.tile([128, 512], f32, tag="ws", bufs=1,
                                  name=f"gbd{b}_{nch}")
                    nc.tensor.matmul(gbd, onesA_sb[0:1, :], den_sb,
                                     start=True, stop=True)
                    gbs = pp.tile([128, 512], f32, tag="ws", bufs=1,
                                  name=f"gbs{b}_{nch}")
                    nc.tensor.matmul(gbs, onesA_sb[0:1, :], sel_sb,
                                     start=True, stop=True)
                    recd = sp.tile([128, 512], f32, tag="recd", bufs=1,
                                   name=f"recd{b}_{nch}")
                    nc.vector.reciprocal(recd, gbd)
                    nc.vector.tensor_mul(g_bc[:, ts(nch, 512)], gbs, recd)
                # ---- FFN expert ----
                for tt in range(NTT):
                    y_ps = pp.tile([128, DC, TT], f32, tag="y", bufs=1,
                                   name=f"yps{b}_{tt}")

                    def emit_h(hc, tt=tt):
                        hp = pp.tile([128, TT], f32, tag="h", bufs=2,
                                     name=f"hps{b}_{tt}_{hc}")
                        for d in range(DC):
                            nc.tensor.matmul(
                                hp,
                                w1_sb[:, d, ts(hc, 128)],
                                oT[:, d, ts(tt, TT)],
                                start=(d == 0), stop=(d == DC - 1))
                        return hp

                    hp = emit_h(0)
                    for hc in range(HC):
                        hT = sp.tile([128, TT], bf16, tag="hT", bufs=3,
                                     name=f"hT{b}_{tt}_{hc}")
                        nc.scalar.activation(hT, hp, AF.Gelu_apprx_tanh,
                                             bias=b1_sb[:, hc:hc + 1],
                                             scale=1.0)
                        if hc + 1 < HC:
                            hp = emit_h(hc + 1)
                        for d2 in range(DC):
                            ymm = nc.tensor.matmul(
                                y_ps[:, d2, :],
                                w2_sb[:, hc, ts(d2, 128)],
                                hT,
                                start=(hc == 0 and d2 % 2 == 0),
                                stop=(hc == HC - 1 and d2 % 2 == 1),
                                skip_group_check=(d2 % 2 == 1))
                            if hc == 0 and d2 % 2 == 0:
                                ylast = ymm
                            elif hc == 0 and d2 % 2 == 1:
                                dep(ymm, ylast, "y odd zero after even start")
                            if hc == HC - 1 and d2 % 2 == 0:
                                ylast = ymm
                            elif hc == HC - 1 and d2 % 2 == 1:
                                dep(ymm, ylast, "y stop after even last")
                    for d2 in range(DC):
                        y_bf = sp.tile([128, TT], bf16, tag="y_bf", bufs=3,
                                       name=f"ybf{b}_{tt}_{d2}")
                        nc.vector.scalar_tensor_tensor(
                            y_bf, y_ps[:, d2, :], b2_sb[:, d2:d2 + 1],
                            g_bc[:, ts(tt, TT)],
                            op0=OP.add, op1=OP.mult)
                        nc.sync.dma_start(rsin[b][d2, :, ts(tt, TT)], y_bf)
                nc.gpsimd.collective_compute(
                    "ReduceScatter", OP.add, replica_groups=rg,
                    ins=[rsin[b].opt()], outs=[rsout[b].opt()])
                for nch in range(2):
                    ob_bf = sp.tile([128, 512], bf16, tag="ob_bf", bufs=1,
                                    name=f"obbf{b}_{nch}")
                    nc.sync.dma_start(ob_bf, rsout[b][:, ts(nch, 512)])
                    ob = sp.tile([128, 512], f32, tag="ob", bufs=1,
                                 name=f"ob{b}_{nch}")
                    nc.vector.tensor_copy(ob, ob_bf)
                    nc.sync.dma_start(out_e[b, :, ts(nch, 512)], ob)

            # software-pipelined emission: attn runs one batch ahead of moe
            if debug_attn:
                emit_attention(0)
            else:
                emit_attention(0)
                load_moe_weights()
                for b in range(1, B):
                    emit_attention(b)
                    emit_moe(b - 1)
                emit_moe(B - 1)

    nc.compile()
    return nc


def _prep_inputs(inputs):
    """Host-side shard prep: slice/transpose/cast per core."""
    f32 = np.float32

    def b(x):
        return np.ascontiguousarray(x).astype(bf16np)

    x = inputs["x"].astype(f32)
    w_qkv = inputs["w_qkv"].astype(f32)
    w_out = inputs["w_out"].astype(f32)
    qn_g, kn_g = inputs["qn_g"].astype(f32), inputs["kn_g"].astype(f32)
    gate_w, gate_b = inputs["gate_w"].astype(f32), inputs["gate_b"].astype(f32)
    w1, b1, w2, b2 = (inputs["w1"].astype(f32), inputs["b1"].astype(f32),
                      inputs["w2"].astype(f32), inputs["b2"].astype(f32))

    xT = b(x.transpose(0, 2, 1).reshape(B, DC, 128, S))
    gatewr = b(gate_w.reshape(DC, 128, E))
    ii, jj = np.meshgrid(np.arange(128), np.arange(256), indexing="ij")
    cmaskT2a = np.where(ii <= jj, 0.0, NEG).astype(f32)   # diag on left half
    cmaskT2b = np.where(jj < 128, NEG,
                        np.where(ii <= jj - 128, 0.0, NEG)).astype(f32)
    ii, jj = np.meshgrid(np.arange(128), np.arange(128), indexing="ij")
    mmean = b(np.where(ii // 64 == jj // 64, 1.0 / 64, 0.0).astype(f32))
    onesA = np.ones((2, 128), bf16np)
    qg = np.tile(qn_g, 2).reshape(128, 1).astype(f32)
    kg = np.tile(kn_g, 2).reshape(128, 1).astype(f32)
    gateb = gate_b.reshape(E, 1).astype(f32)

    in_maps = []
    for c in range(NC):
        h0, h1 = HPC * c, HPC * c + 1
        cs = np.r_[h0 * 64:(h0 + 1) * 64, h1 * 64:(h1 + 1) * 64]
        wqk_c = np.concatenate(
            [w_qkv[:, cs], w_qkv[:, DIM + cs]], axis=1)          # [1024,256]
        wv_c = w_qkv[:, 2 * DIM + cs]                            # [1024,128]
        wout_c = w_out[cs, :]                                    # [128,1024]
        onehot = np.zeros((E, 1), bf16np)
        onehot[c, 0] = 1.0
        in_maps.append({
            "xT": xT,
            "wqk": b(wqk_c.reshape(DC, 128, 256)),
            "wv": b(wv_c.reshape(DC, 128, 128)),
            "wout": b(wout_c.reshape(128, DC, 128).transpose(1, 0, 2)),
            "w1": b(w1[c].reshape(DC, 128, HID)),
            "w2": b(w2[c].reshape(HC, 128, DIM)),
            "gatew": gatewr,
            "b1s": np.ascontiguousarray(b1[c].reshape(HC, 128).T).astype(f32),
            "b2s": np.ascontiguousarray(b2[c].reshape(DC, 128).T).astype(f32),
            "gateb": gateb,
            "qg": qg,
            "kg": kg,
            "cmaskT2a": cmaskT2a,
            "cmaskT2b": cmaskT2b,
            "ones128k": np.ones((128, 1), bf16np),
            "onesB": np.ones((128, 64), bf16np),
            "mmean": mmean,
            "ones8": np.ones((E, 1), bf16np),
            "onehot": onehot,
            "onesA": onesA,
        })
    return in_maps


def kernel(**inputs):
    from concourse.bass_utils import run_bass_kernel_spmd

    if "nc" not in _cache:
        _cache["nc"] = _build()
    nc = _cache["nc"]
    in_maps = _prep_inputs(inputs)
    res = run_bass_kernel_spmd(nc, in_maps, core_ids=list(range(NC)))
    full = np.empty((B, S, DIM), np.float32)
    for c in range(NC):
        full[:, :, 128 * c:128 * (c + 1)] = (
            res.results[c]["out"].transpose(0, 2, 1))
    return full


# revision 25
# speedup vs baseline: 1.0891x; 1.0122x over previous
"""Trainium2 Bass kernel for a dense-MoE encoder layer (8 NeuronCores).

Sharding:
  - Attention: head-parallel (16 heads / 8 cores = 2 heads per core); the
    output-projection partial sums are AllReduced (bf16) per batch.
  - MoE: expert-parallel (8 experts / 8 cores = 1 expert per core); the
    gate-weighted expert outputs are ReduceScattered (bf16) per batch.

All matmuls run in bf16 with fp32 PSUM accumulation. Softmax skips the
max-subtraction (qk-norm bounds |score*scale| <= 8). Host-side prep does
layout/transpose/dtype conversion only. Emission is software-pipelined:
attn(b+1) is emitted between attn(b) and moe(b) so each batch's AllReduce
overlaps the next batch's attention and the previous batch's MoE.
"""

import numpy as np
import ml_dtypes

B, S, DIM, HEADS, DH = 4, 1024, 1024, 16, 64
E, HID = 8, 4096
NC = 8
HPC = HEADS // NC  # heads per core
SCALE = DH ** -0.5
EPS = 1e-5
NEG = -30000.0
TT = 256  # MoE token tile
NTT = S // TT
DC = DIM // 128  # 8 d-chunks
HC = HID // 128  # 32 hid-chunks

bf16np = ml_dtypes.bfloat16

_cache = {}


def _build(debug_attn=False):
    import concourse.mybir as mybir
    import concourse.tile as tile
    from concourse import bacc
    from concourse.bass import ts

    bf16 = mybir.dt.bfloat16
    f32 = mybir.dt.float32
    AF = mybir.ActivationFunctionType
    OP = mybir.AluOpType

    import bass_rust

    def dep(a, b, why):
        # a must run after b (same-engine ordering for PSUM zero-region tricks)
        bass_rust.add_dep_helper(a.ins, b.ins, reason=why)

    nc = bacc.Bacc(None, target_bir_lowering=False, debug=False, num_devices=NC)
    P = {}
    for name, shape, dt in [
        ("xT", [B, DC, 128, S], bf16),
        ("wqk", [DC, 128, 2 * 128], bf16),
        ("wv", [DC, 128, 128], bf16),
        ("wout", [DC, 128, 128], bf16),
        ("w1", [DC, 128, HID], bf16),
        ("w2", [HC, 128, DIM], bf16),
        ("gatew", [DC, 128, E], bf16),
        ("b1s", [128, HC], f32),
        ("b2s", [128, DC], f32),
        ("gateb", [E, 1], f32),
        ("qg", [128, 1], f32),
        ("kg", [128, 1], f32),
        ("cmaskT2a", [128, 256], f32),
        ("cmaskT2b", [128, 256], f32),
        ("ones128k", [128, 1], bf16),
        ("onesB", [128, 64], bf16),
        ("mmean", [128, 128], bf16),
        ("ones8", [E, 1], bf16),
        ("onehot", [E, 1], bf16),
        ("onesA", [2, 128], bf16),
    ]:
        P[name] = nc.declare_dram_parameter(name, shape, dt, isOutput=False)
    out_e = nc.declare_dram_parameter("out", [B, 128, S], f32, isOutput=True)

    rg = [list(range(NC))]

    with tile.TileContext(nc, num_cores=NC) as tc:
        with (
            tc.tile_pool(name="wp", bufs=1) as wp,
            tc.tile_pool(name="sp", bufs=2) as sp,
            tc.tile_pool(name="pp", bufs=1, space="PSUM") as pp,
            tc.tile_pool(name="dp", bufs=1, space="DRAM") as dp,
        ):
            # ---- resident weights / constants ----
            w1_sb = wp.tile([128, DC, HID], bf16)
            w2_sb = wp.tile([128, HC, DIM], bf16)
            wqk_sb = wp.tile([128, DC, 256], bf16)
            wv_sb = wp.tile([128, DC, 128], bf16)
            wout_sb = wp.tile([128, DC, 128], bf16)
            gatew_sb = wp.tile([128, DC, E], bf16)
            for d in range(DC):
                nc.sync.dma_start(wqk_sb[:, d, :], P["wqk"][d])
                nc.sync.dma_start(wv_sb[:, d, :], P["wv"][d])
                nc.sync.dma_start(wout_sb[:, d, :], P["wout"][d])

            def load_moe_weights():
                for d in range(DC):
                    nc.sync.dma_start(w1_sb[:, d, :], P["w1"][d])
                    nc.sync.dma_start(gatew_sb[:, d, :], P["gatew"][d])
                for h in range(HC):
                    nc.sync.dma_start(w2_sb[:, h, :], P["w2"][h])
            b1_sb = wp.tile([128, HC], f32)
            b2_sb = wp.tile([128, DC], f32)
            gateb_sb = wp.tile([E, 1], f32)
            qg_sb = wp.tile([128, 1], f32)
            kg_sb = wp.tile([128, 1], f32)
            cm2a_sb = wp.tile([128, 256], f32)
            cm2b_sb = wp.tile([128, 256], f32)
            ones128k_sb = wp.tile([128, 1], bf16)
            onesB_sb = wp.tile([128, 64], bf16)
            mmean_sb = wp.tile([128, 128], bf16)
            ones8_sb = wp.tile([E, 1], bf16)
            onehot_sb = wp.tile([E, 1], bf16)
            onesA_sb = wp.tile([2, 128], bf16)
            eps_sb = wp.tile([128, 1], f32)
            for nm, t in [
                ("b1s", b1_sb), ("b2s", b2_sb), ("gateb", gateb_sb),
                ("qg", qg_sb), ("kg", kg_sb),
                ("cmaskT2a", cm2a_sb), ("cmaskT2b", cm2b_sb),
                ("ones128k", ones128k_sb), ("onesB", onesB_sb),
                ("mmean", mmean_sb),
                ("ones8", ones8_sb), ("onehot", onehot_sb), ("onesA", onesA_sb),
            ]:
                nc.sync.dma_start(t, P[nm][:, :])
            nc.vector.memset(eps_sb, EPS)

            arin, arout, rsin, rsout = [], [], [], []
            for b in range(B):
                arin.append([dp.tile([DC, 128, 512], bf16,
                                     name=f"arin{b}_{x}") for x in range(2)])
                arout.append([dp.tile([DC, 128, 512], bf16,
                                      name=f"arout{b}_{x}",
                                      addr_space="Shared") for x in range(2)])
                rsin.append([dp.tile([DC, 128, 512], bf16,
                                     name=f"rsin{b}_{x}") for x in range(2)])
                rsout.append([dp.tile([128, 512], bf16,
                                      name=f"rsout{b}_{x}") for x in range(2)])

            # PSUM tags (8 banks total):
            #   y: [128,8,256] f32 = 4 banks, bufs=1
            #   h: [128,256] f32 = 1 bank, bufs=2 (double-buffered h GEMM)
            #   ws: [128,512] f32 = 1 bank, bufs=1 (all transient matmul outs)
            #   pv: [128,512] f32 = 1 bank, bufs=1 (PV accum + softmax denom)

            def emit_attention(b):
                # ---- qkv projection (feature-major q,k) + qk-norm ----
                qkT = sp.tile([128, 2, S], bf16, tag="qkT", bufs=2,
                              name=f"qkT{b}")
                for nch in range(2):
                    for m in range(2):
                        qk_ps = pp.tile([128, 512], f32, tag="ws", bufs=1,
                                        name=f"qkps{b}_{nch}_{m}")
                        for d in range(DC):
                            xt = sp.tile([128, 512], bf16, tag="xt", bufs=2,
                                         name=f"xt{b}_{nch}_{m}_{d}")
                            nc.sync.dma_start(xt,
                                              P["xT"][b, d, :, ts(nch, 512)])
                            nc.tensor.matmul(
                                qk_ps, wqk_sb[:, d, ts(m, 128)], xt,
                                start=(d == 0), stop=(d == DC - 1))
                        nc.scalar.copy(qkT[:, m, ts(nch, 512)], qk_ps)
                    for m in range(2):
                        qraw = qkT[:, m, ts(nch, 512)]
                        mu_ps = pp.tile([128, 512], f32, tag="ws", bufs=1,
                                        name=f"mups{b}_{nch}_{m}")
                        nc.tensor.matmul(mu_ps, mmean_sb, qraw,
                                         start=True, stop=True)
                        sub = sp.tile([128, 512], bf16, tag="sub",
                                      name=f"sub{b}_{nch}_{m}")
                        nc.vector.tensor_tensor(sub, qraw, mu_ps, OP.subtract)
                        sq = sp.tile([128, 512], bf16, tag="sq",
                                     name=f"sq{b}_{nch}_{m}")
                        nc.vector.tensor_mul(sq, sub, sub)
                        var_ps = pp.tile([128, 512], f32, tag="ws", bufs=1,
                                         name=f"varps{b}_{nch}_{m}")
                        nc.tensor.matmul(var_ps, mmean_sb, sq,
                                         start=True, stop=True)
                        rstd = sp.tile([128, 512], f32, tag="rstd",
                                       name=f"rstd{b}_{nch}_{m}")
                        nc.scalar.activation(rstd, var_ps, AF.Sqrt,
                                             bias=eps_sb, scale=1.0)
                        nc.vector.reciprocal(rstd, rstd)
                        g_ap = qg_sb if m == 0 else kg_sb
                        nc.vector.scalar_tensor_tensor(
                            qkT[:, m, ts(nch, 512)], sub, g_ap, rstd,
                            op0=OP.mult, op1=OP.mult)
                # ---- v (token-major) ----
                v_sb = sp.tile([128, DC, 128], bf16, tag="v_sb",
                               name=f"vsb{b}")
                for vg in range(2):  # two groups of 4 token-chunks
                    vq = pp.tile([128, 4, 128], f32, tag="ws", bufs=1,
                                 name=f"vq{b}_{vg}")
                    vfirst, vlasts = None, []
                    for tq in range(4):
                        tcn = vg * 4 + tq
                        for d in range(DC):
                            xtv = sp.tile([128, 128], bf16, tag="xtv", bufs=4,
                                          name=f"xtv{b}_{tcn}_{d}")
                            nc.sync.dma_start(
                                xtv, P["xT"][b, d, :, ts(tcn, 128)])
                            mm = nc.tensor.matmul(
                                vq[:, tq, :], xtv, wv_sb[:, d, :],
                                start=(tq == 0 and d == 0),
                                stop=(tq == 3 and d == DC - 1),
                                skip_group_check=(tq != 0))
                            if tq == 0 and d == 0:
                                vfirst = mm
                            elif d == 0:
                                dep(mm, vfirst, "v zero-region after start")
                            if d == DC - 1 and tq < 3:
                                vlasts.append(mm)
                            if tq == 3 and d == DC - 1:
                                for vl in vlasts:
                                    dep(mm, vl, "v stop after all groups")
                        nc.scalar.copy(v_sb[:, tcn, :], vq[:, tq, :])
                # ---- causal attention, 2 heads ----
                oTh = sp.tile([128, S], bf16, tag="oTh", bufs=2,
                              name=f"oTh{b}")
                for qp in range(4):  # 256-query pairs, both heads share pvt
                    pvt = pp.tile([128, 512], f32, tag="pv", bufs=1,
                                  name=f"pvt{b}_{qp}")
                    nkc = 2 * qp + 2
                    for h in range(HPC):
                        hsl = slice(64 * h, 64 * h + 64)
                        dsl = slice(64 * h, 64 * h + 1)
                        pvfirst, pvden = None, None
                        for kc in range(nkc):
                            scT = pp.tile([128, 256], f32, tag="ws", bufs=1,
                                          name=f"scT{b}_{qp}_{h}_{kc}")
                            nc.tensor.matmul(
                                scT,
                                qkT[hsl, 1, ts(kc, 128)],
                                qkT[hsl, 0, ts(qp, 256)],
                                start=True, stop=True)
                            if kc == nkc - 2:
                                nc.vector.tensor_add(scT, scT, cm2a_sb)
                            elif kc == nkc - 1:
                                nc.vector.tensor_add(scT, scT, cm2b_sb)
                            exk = sp.tile([128, 256], bf16, tag="exk", bufs=4,
                                          name=f"exk{b}_{qp}_{h}_{kc}")
                            nc.scalar.activation(exk, scT, AF.Exp, scale=SCALE)
                            pvmm = nc.tensor.matmul(
                                pvt[hsl, 0:256], v_sb[:, kc, hsl], exk,
                                start=(kc == 0), stop=(kc == nkc - 1))
                            if kc == 0:
                                pvfirst = pvmm
                            if kc == nkc - 1 and pvden is not None:
                                dep(pvmm, pvden, "pv stop after last den")
                            dmm = nc.tensor.matmul(
                                pvt[dsl, 256:512], ones128k_sb, exk,
                                start=False, stop=False, skip_group_check=True)
                            if kc == 0:
                                dep(dmm, pvfirst, "den zero after pv start")
                            pvden = dmm
                    for h in range(HPC):
                        hsl = slice(64 * h, 64 * h + 64)
                        dsl = slice(64 * h, 64 * h + 1)
                        rec_sb = sp.tile([128, 256], f32, tag="rec_sb",
                                         bufs=2, name=f"rcs{b}_{qp}_{h}")
                        nc.vector.reciprocal(rec_sb[dsl, :],
                                             pvt[dsl, 256:512])
                        rec_bf = sp.tile([128, 256], bf16, tag="rec_bf",
                                         bufs=2, name=f"rcb{b}_{qp}_{h}")
                        nc.vector.tensor_copy(rec_bf[dsl, :], rec_sb[dsl, :])
                        rb = pp.tile([128, 256], f32, tag="ws", bufs=1,
                                     name=f"rb{b}_{qp}_{h}")
                        nc.tensor.matmul(rb[hsl, :], onesB_sb[dsl, :],
                                         rec_bf[dsl, :],
                                         start=True, stop=True)
                        rb_sb = sp.tile([128, 256], f32, tag="rb_sb",
                                        bufs=2, name=f"rbs{b}_{qp}_{h}")
                        nc.scalar.copy(rb_sb[hsl, :], rb[hsl, :])
                        nc.vector.tensor_mul(oTh[hsl, ts(qp, 256)],
                                             pvt[hsl, 0:256], rb_sb[hsl, :])
                    if qp % 2 == 1:
                        nch = qp // 2
                        for mc in range(DC):
                            wo_ps = pp.tile([128, 512], f32, tag="ws", bufs=1,
                                            name=f"wops{b}_{mc}_{nch}")
                            nc.tensor.matmul(wo_ps, wout_sb[:, mc, :],
                                             oTh[:, ts(nch, 512)],
                                             start=True, stop=True)
                            wo_bf = sp.tile([128, 512], bf16, tag="wo_bf",
                                            bufs=2, name=f"wobf{b}_{mc}_{nch}")
                            nc.vector.tensor_copy(wo_bf, wo_ps)
                            nc.sync.dma_start(arin[b][nch][mc, :, :], wo_bf)
                        nc.gpsimd.collective_compute(
                            "AllReduce", OP.add, replica_groups=rg,
                            ins=[arin[b][nch].opt()],
                            outs=[arout[b][nch].opt()])


            def emit_moe(b):
                oTx = []
                for x in range(2):
                    t = sp.tile([128, DC, 512], bf16, tag="oT", bufs=2,
                                name=f"oT{b}_{x}")
                    for d in range(DC):
                        nc.sync.dma_start(t[:, d, :], arout[b][x][d])
                    oTx.append(t)
                pass
                # ---- gates ----
                g_bc = sp.tile([128, S], bf16, tag="g_bc", bufs=1,
                               name=f"gbc{b}")
                for nch in range(2):
                    lg = pp.tile([E, 512], f32, tag="ws", bufs=1,
                                 name=f"lg{b}_{nch}")
                    for d in range(DC):
                        nc.tensor.matmul(lg, gatew_sb[:, d, :],
                                         oTx[nch][:, d, :],
                                         start=(d == 0), stop=(d == DC - 1))
                    expT = sp.tile([E, 512], bf16, tag="expT",
                                   name=f"expT{b}_{nch}")
                    nc.scalar.activation(expT, lg, AF.Exp, bias=gateb_sb,
                                         scale=1.0)
                    den_ps = pp.tile([1, 512], f32, tag="ws", bufs=1,
                                     name=f"denps{b}_{nch}")
                    nc.tensor.matmul(den_ps, ones8_sb, expT,
                                     start=True, stop=True)
                    sel_ps = pp.tile([1, 512], f32, tag="ws", bufs=1,
                                     name=f"selps{b}_{nch}")
                    nc.tensor.matmul(sel_ps, onehot_sb, expT,
                                     start=True, stop=True)
                    den_sb = sp.tile([1, 512], bf16, tag="den_sb",
                                     name=f"densb{b}_{nch}")
                    nc.scalar.copy(den_sb, den_ps)
                    sel_sb = sp.tile([1, 512], bf16, tag="sel_sb",
                                     name=f"selsb{b}_{nch}")
                    nc.scalar.copy(sel_sb, sel_ps)
                    # broadcast denominator and selected-expert rows to 128
                    # partitions via K=1 matmuls, then g = sel * (1/den)
                    gbd = pp.tile([128, 512], f32, tag="ws", bufs=1,
                                  name=f"gbd{b}_{nch}")
                    nc.tensor.matmul(gbd, onesA_sb[0:1, :], den_sb,
                                     start=True, stop=True)
                    gbs = pp.tile([128, 512], f32, tag="ws", bufs=1,
                                  name=f"gbs{b}_{nch}")
                    nc.tensor.matmul(gbs, onesA_sb[0:1, :], sel_sb,
                                     start=True, stop=True)
                    recd = sp.tile([128, 512], f32, tag="recd", bufs=1,
                                   name=f"recd{b}_{nch}")
                    nc.vector.reciprocal(recd, gbd)
                    nc.vector.tensor_mul(g_bc[:, ts(nch, 512)], gbs, recd)
                # ---- FFN expert ----
                for tt in range(NTT):
                    y_ps = pp.tile([128, DC, TT], f32, tag="y", bufs=1,
                                   name=f"yps{b}_{tt}")

                    def emit_h(hc, tt=tt):
                        hp = pp.tile([128, TT], f32, tag="h", bufs=2,
                                     name=f"hps{b}_{tt}_{hc}")
                        for d in range(DC):
                            nc.tensor.matmul(
                                hp,
                                w1_sb[:, d, ts(hc, 128)],
                                oTx[tt // 2][:, d, ts(tt % 2, TT)],
                                start=(d == 0), stop=(d == DC - 1))
                        return hp

                    hp = emit_h(0)
                    for hc in range(HC):
                        hT = sp.tile([128, TT], bf16, tag="hT", bufs=4,
                                     name=f"hT{b}_{tt}_{hc}")
                        nc.scalar.activation(hT, hp, AF.Gelu_apprx_tanh,
                                             bias=b1_sb[:, hc:hc + 1],
                                             scale=1.0)
                        if hc + 1 < HC:
                            hp = emit_h(hc + 1)
                        for d2 in range(DC):
                            ymm = nc.tensor.matmul(
                                y_ps[:, d2, :],
                                w2_sb[:, hc, ts(d2, 128)],
                                hT,
                                start=(hc == 0 and d2 % 2 == 0),
                                stop=(hc == HC - 1 and d2 % 2 == 1),
                                skip_group_check=(d2 % 2 == 1))
                            if hc == 0 and d2 % 2 == 0:
                                ylast = ymm
                            elif hc == 0 and d2 % 2 == 1:
                                dep(ymm, ylast, "y odd zero after even start")
                            if hc == HC - 1 and d2 % 2 == 0:
                                ylast = ymm
                            elif hc == HC - 1 and d2 % 2 == 1:
                                dep(ymm, ylast, "y stop after even last")
                    for d2 in range(DC):
                        y_bf = sp.tile([128, TT], bf16, tag="y_bf", bufs=3,
                                       name=f"ybf{b}_{tt}_{d2}")
                        nc.vector.scalar_tensor_tensor(
                            y_bf, y_ps[:, d2, :], b2_sb[:, d2:d2 + 1],
                            g_bc[:, ts(tt, TT)],
                            op0=OP.add, op1=OP.mult)
                        nc.sync.dma_start(
                            rsin[b][tt // 2][d2, :, ts(tt % 2, TT)], y_bf)
                    if tt % 2 == 1:
                        x = tt // 2
                        nc.gpsimd.collective_compute(
                            "ReduceScatter", OP.add, replica_groups=rg,
                            ins=[rsin[b][x].opt()], outs=[rsout[b][x].opt()])
                        ob_bf = sp.tile([128, 512], bf16, tag="ob_bf", bufs=1,
                                        name=f"obbf{b}_{x}")
                        nc.sync.dma_start(ob_bf, rsout[b][x][:, :])
                        ob = sp.tile([128, 512], f32, tag="ob", bufs=1,
                                     name=f"ob{b}_{x}")
                        nc.vector.tensor_copy(ob, ob_bf)
                        nc.sync.dma_start(out_e[b, :, ts(x, 512)], ob)
                pass

            # software-pipelined emission: attn runs one batch ahead of moe
            if debug_attn:
                emit_attention(0)
            else:
                emit_attention(0)
                load_moe_weights()
                for b in range(1, B):
                    emit_attention(b)
                    emit_moe(b - 1)
                emit_moe(B - 1)

    nc.compile()
    return nc


def _prep_inputs(inputs):
    """Host-side shard prep: slice/transpose/cast per core."""
    f32 = np.float32

    def b(x):
        return np.ascontiguousarray(x).astype(bf16np)

    x = inputs["x"].astype(f32)
    w_qkv = inputs["w_qkv"].astype(f32)
    w_out = inputs["w_out"].astype(f32)
    qn_g, kn_g = inputs["qn_g"].astype(f32), inputs["kn_g"].astype(f32)
    gate_w, gate_b = inputs["gate_w"].astype(f32), inputs["gate_b"].astype(f32)
    w1, b1, w2, b2 = (inputs["w1"].astype(f32), inputs["b1"].astype(f32),
                      inputs["w2"].astype(f32), inputs["b2"].astype(f32))

    xT = b(x.transpose(0, 2, 1).reshape(B, DC, 128, S))
    gatewr = b(gate_w.reshape(DC, 128, E))
    ii, jj = np.meshgrid(np.arange(128), np.arange(256), indexing="ij")
    cmaskT2a = np.where(ii <= jj, 0.0, NEG).astype(f32)   # diag on left half
    cmaskT2b = np.where(jj < 128, NEG,
                        np.where(ii <= jj - 128, 0.0, NEG)).astype(f32)
    ii, jj = np.meshgrid(np.arange(128), np.arange(128), indexing="ij")
    mmean = b(np.where(ii // 64 == jj // 64, 1.0 / 64, 0.0).astype(f32))
    onesA = np.ones((2, 128), bf16np)
    qg = np.tile(qn_g, 2).reshape(128, 1).astype(f32)
    kg = np.tile(kn_g, 2).reshape(128, 1).astype(f32)
    gateb = gate_b.reshape(E, 1).astype(f32)

    in_maps = []
    for c in range(NC):
        h0, h1 = HPC * c, HPC * c + 1
        cs = np.r_[h0 * 64:(h0 + 1) * 64, h1 * 64:(h1 + 1) * 64]
        wqk_c = np.concatenate(
            [w_qkv[:, cs], w_qkv[:, DIM + cs]], axis=1)          # [1024,256]
        wv_c = w_qkv[:, 2 * DIM + cs]                            # [1024,128]
        wout_c = w_out[cs, :]                                    # [128,1024]
        onehot = np.zeros((E, 1), bf16np)
        onehot[c, 0] = 1.0
        in_maps.append({
            "xT": xT,
            "wqk": b(wqk_c.reshape(DC, 128, 256)),
            "wv": b(wv_c.reshape(DC, 128, 128)),
            "wout": b(wout_c.reshape(128, DC, 128).transpose(1, 0, 2)),
            "w1": b(w1[c].reshape(DC, 128, HID)),
            "w2": b(w2[c].reshape(HC, 128, DIM)),
            "gatew": gatewr,
            "b1s": np.ascontiguousarray(b1[c].reshape(HC, 128).T).astype(f32),
            "b2s": np.ascontiguousarray(b2[c].reshape(DC, 128).T).astype(f32),
            "gateb": gateb,
            "qg": qg,
            "kg": kg,
            "cmaskT2a": cmaskT2a,
            "cmaskT2b": cmaskT2b,
            "ones128k": np.ones((128, 1), bf16np),
            "onesB": np.ones((128, 64), bf16np),
            "mmean": mmean,
            "ones8": np.ones((E, 1), bf16np),
            "onehot": onehot,
            "onesA": onesA,
        })
    return in_maps


def kernel(**inputs):
    from concourse.bass_utils import run_bass_kernel_spmd

    if "nc" not in _cache:
        _cache["nc"] = _build()
    nc = _cache["nc"]
    in_maps = _prep_inputs(inputs)
    res = run_bass_kernel_spmd(nc, in_maps, core_ids=list(range(NC)))
    full = np.empty((B, S, DIM), np.float32)
    for c in range(NC):
        full[:, :, 128 * c:128 * (c + 1)] = (
            res.results[c]["out"].transpose(0, 2, 1))
    return full


# revision 26
# speedup vs baseline: 1.1991x; 1.1010x over previous
"""Trainium2 Bass kernel for a dense-MoE encoder layer (8 NeuronCores).

Sharding:
  - Attention: head-parallel (16 heads / 8 cores = 2 heads per core); the
    output-projection partial sums are AllReduced (bf16) per batch.
  - MoE: expert-parallel (8 experts / 8 cores = 1 expert per core); the
    gate-weighted expert outputs are ReduceScattered (bf16) per batch.

All matmuls run in bf16 with fp32 PSUM accumulation. Softmax skips the
max-subtraction (qk-norm bounds |score*scale| <= 8). Host-side prep does
layout/transpose/dtype conversion only. Emission is software-pipelined:
attn(b+1) is emitted between attn(b) and moe(b) so each batch's AllReduce
overlaps the next batch's attention and the previous batch's MoE.
"""

import numpy as np
import ml_dtypes

B, S, DIM, HEADS, DH = 4, 1024, 1024, 16, 64
E, HID = 8, 4096
NC = 8
HPC = HEADS // NC  # heads per core
SCALE = DH ** -0.5
EPS = 1e-5
NEG = -30000.0
TT = 256  # MoE token tile
NTT = S // TT
DC = DIM // 128  # 8 d-chunks
HC = HID // 128  # 32 hid-chunks

bf16np = ml_dtypes.bfloat16

_cache = {}


def _build(debug_attn=False):
    import concourse.mybir as mybir
    import concourse.tile as tile
    from concourse import bacc
    from concourse.bass import ts

    bf16 = mybir.dt.bfloat16
    f32 = mybir.dt.float32
    AF = mybir.ActivationFunctionType
    OP = mybir.AluOpType

    import bass_rust

    def dep(a, b, why):
        # a must run after b (same-engine ordering for PSUM zero-region tricks)
        bass_rust.add_dep_helper(a.ins, b.ins, reason=why)

    nc = bacc.Bacc(None, target_bir_lowering=False, debug=False, num_devices=NC)
    P = {}
    for name, shape, dt in [
        ("xT", [B, DC, 128, S], bf16),
        ("wqk", [DC, 128, 2 * 128], bf16),
        ("wv", [DC, 128, 128], bf16),
        ("wout", [DC, 128, 128], bf16),
        ("w1", [DC, 128, HID], bf16),
        ("w2", [HC, 128, DIM], bf16),
        ("gatew", [DC, 128, E], bf16),
        ("b1s", [128, HC], f32),
        ("b2s", [128, DC], f32),
        ("gateb", [E, 1], f32),
        ("qg", [128, 1], f32),
        ("kg", [128, 1], f32),
        ("cmaskT2a", [128, 256], f32),
        ("cmaskT2b", [128, 256], f32),
        ("ones128k", [128, 1], bf16),
        ("onesB", [128, 64], bf16),
        ("mmean", [128, 128], bf16),
        ("ones8", [E, 1], bf16),
        ("onehot", [E, 1], bf16),
        ("onesA", [2, 128], bf16),
    ]:
        P[name] = nc.declare_dram_parameter(name, shape, dt, isOutput=False)
    out_e = nc.declare_dram_parameter("out", [B, 128, S], f32, isOutput=True)

    rg = [list(range(NC))]

    with tile.TileContext(nc, num_cores=NC) as tc:
        with (
            tc.tile_pool(name="wp", bufs=1) as wp,
            tc.tile_pool(name="sp", bufs=2) as sp,
            tc.tile_pool(name="pp", bufs=1, space="PSUM") as pp,
            tc.tile_pool(name="dp", bufs=1, space="DRAM") as dp,
        ):
            # ---- resident weights / constants ----
            w1_sb = wp.tile([128, DC, HID], bf16)
            w2_sb = wp.tile([128, HC, DIM], bf16)
            wqk_sb = wp.tile([128, DC, 256], bf16)
            wv_sb = wp.tile([128, DC, 128], bf16)
            wout_sb = wp.tile([128, DC, 128], bf16)
            gatew_sb = wp.tile([128, DC, E], bf16)
            for d in range(DC):
                nc.sync.dma_start(wqk_sb[:, d, :], P["wqk"][d])
                nc.sync.dma_start(wv_sb[:, d, :], P["wv"][d])
                nc.sync.dma_start(wout_sb[:, d, :], P["wout"][d])

            def load_moe_weights():
                for d in range(DC):
                    nc.sync.dma_start(w1_sb[:, d, :], P["w1"][d])
                    nc.sync.dma_start(gatew_sb[:, d, :], P["gatew"][d])
                for h in range(HC):
                    nc.sync.dma_start(w2_sb[:, h, :], P["w2"][h])
            b1_sb = wp.tile([128, HC], f32)
            b2_sb = wp.tile([128, DC], f32)
            gateb_sb = wp.tile([E, 1], f32)
            qg_sb = wp.tile([128, 1], f32)
            kg_sb = wp.tile([128, 1], f32)
            cm2a_sb = wp.tile([128, 256], f32)
            cm2b_sb = wp.tile([128, 256], f32)
            ones128k_sb = wp.tile([128, 1], bf16)
            onesB_sb = wp.tile([128, 64], bf16)
            mmean_sb = wp.tile([128, 128], bf16)
            ones8_sb = wp.tile([E, 1], bf16)
            onehot_sb = wp.tile([E, 1], bf16)
            onesA_sb = wp.tile([2, 128], bf16)
            eps_sb = wp.tile([128, 1], f32)
            for nm, t in [
                ("b1s", b1_sb), ("b2s", b2_sb), ("gateb", gateb_sb),
                ("qg", qg_sb), ("kg", kg_sb),
                ("cmaskT2a", cm2a_sb), ("cmaskT2b", cm2b_sb),
                ("ones128k", ones128k_sb), ("onesB", onesB_sb),
                ("mmean", mmean_sb),
                ("ones8", ones8_sb), ("onehot", onehot_sb), ("onesA", onesA_sb),
            ]:
                nc.sync.dma_start(t, P[nm][:, :])
            nc.vector.memset(eps_sb, EPS)

            arin, arout, rsin, rsout = [], [], [], []
            for b in range(B):
                arin.append([dp.tile([DC, 128, 512], bf16,
                                     name=f"arin{b}_{x}") for x in range(2)])
                arout.append([dp.tile([DC, 128, 512], bf16,
                                      name=f"arout{b}_{x}",
                                      addr_space="Shared") for x in range(2)])
                rsin.append([dp.tile([DC, 128, 512], bf16,
                                     name=f"rsin{b}_{x}") for x in range(2)])
                rsout.append([dp.tile([128, 512], bf16,
                                      name=f"rsout{b}_{x}") for x in range(2)])

            # PSUM tags (8 banks total):
            #   y: [128,8,256] f32 = 4 banks, bufs=1
            #   h: [128,256] f32 = 1 bank, bufs=2 (double-buffered h GEMM)
            #   ws: [128,512] f32 = 1 bank, bufs=1 (all transient matmul outs)
            #   pv: [128,512] f32 = 1 bank, bufs=1 (PV accum + softmax denom)

            def emit_attention(b):
                # ---- qkv projection (feature-major q,k) + qk-norm ----
                qkT = sp.tile([128, 2, S], bf16, tag="qkT", bufs=2,
                              name=f"qkT{b}")
                for nch in range(2):
                    for m in range(2):
                        qk_ps = pp.tile([128, 512], f32, tag="ws", bufs=2,
                                        name=f"qkps{b}_{nch}_{m}")
                        for d in range(DC):
                            xt = sp.tile([128, 512], bf16, tag="xt", bufs=2,
                                         name=f"xt{b}_{nch}_{m}_{d}")
                            nc.sync.dma_start(xt,
                                              P["xT"][b, d, :, ts(nch, 512)])
                            nc.tensor.matmul(
                                qk_ps, wqk_sb[:, d, ts(m, 128)], xt,
                                start=(d == 0), stop=(d == DC - 1))
                        nc.scalar.copy(qkT[:, m, ts(nch, 512)], qk_ps)
                    for m in range(2):
                        qraw = qkT[:, m, ts(nch, 512)]
                        mu_ps = pp.tile([128, 512], f32, tag="ws", bufs=2,
                                        name=f"mups{b}_{nch}_{m}")
                        nc.tensor.matmul(mu_ps, mmean_sb, qraw,
                                         start=True, stop=True)
                        sub = sp.tile([128, 512], bf16, tag="sub",
                                      name=f"sub{b}_{nch}_{m}")
                        nc.vector.tensor_tensor(sub, qraw, mu_ps, OP.subtract)
                        sq = sp.tile([128, 512], bf16, tag="sq",
                                     name=f"sq{b}_{nch}_{m}")
                        nc.vector.tensor_mul(sq, sub, sub)
                        var_ps = pp.tile([128, 512], f32, tag="ws", bufs=2,
                                         name=f"varps{b}_{nch}_{m}")
                        nc.tensor.matmul(var_ps, mmean_sb, sq,
                                         start=True, stop=True)
                        rstd = sp.tile([128, 512], f32, tag="rstd",
                                       name=f"rstd{b}_{nch}_{m}")
                        nc.scalar.activation(rstd, var_ps, AF.Sqrt,
                                             bias=eps_sb, scale=1.0)
                        nc.vector.reciprocal(rstd, rstd)
                        g_ap = qg_sb if m == 0 else kg_sb
                        nc.vector.scalar_tensor_tensor(
                            qkT[:, m, ts(nch, 512)], sub, g_ap, rstd,
                            op0=OP.mult, op1=OP.mult)
                # ---- v (token-major) ----
                v_sb = sp.tile([128, DC, 128], bf16, tag="v_sb",
                               name=f"vsb{b}")
                for vg in range(2):  # two groups of 4 token-chunks
                    vq = pp.tile([128, 4, 128], f32, tag="ws", bufs=2,
                                 name=f"vq{b}_{vg}")
                    vfirst, vlasts = None, []
                    for tq in range(4):
                        tcn = vg * 4 + tq
                        for d in range(DC):
                            xtv = sp.tile([128, 128], bf16, tag="xtv", bufs=4,
                                          name=f"xtv{b}_{tcn}_{d}")
                            nc.sync.dma_start(
                                xtv, P["xT"][b, d, :, ts(tcn, 128)])
                            mm = nc.tensor.matmul(
                                vq[:, tq, :], xtv, wv_sb[:, d, :],
                                start=(tq == 0 and d == 0),
                                stop=(tq == 3 and d == DC - 1),
                                skip_group_check=(tq != 0))
                            if tq == 0 and d == 0:
                                vfirst = mm
                            elif d == 0:
                                dep(mm, vfirst, "v zero-region after start")
                            if d == DC - 1 and tq < 3:
                                vlasts.append(mm)
                            if tq == 3 and d == DC - 1:
                                for vl in vlasts:
                                    dep(mm, vl, "v stop after all groups")
                        nc.scalar.copy(v_sb[:, tcn, :], vq[:, tq, :])
                # ---- causal attention, 2 heads ----
                oTh = sp.tile([128, S], bf16, tag="oTh", bufs=2,
                              name=f"oTh{b}")
                for qp in range(4):  # 256-query pairs, both heads share pvt
                    pvt = pp.tile([128, 512], f32, tag="pv", bufs=1,
                                  name=f"pvt{b}_{qp}")
                    nkc = 2 * qp + 2
                    for h in range(HPC):
                        hsl = slice(64 * h, 64 * h + 64)
                        dsl = slice(64 * h, 64 * h + 1)
                        pvfirst, pvden = None, None
                        for kc in range(nkc):
                            scT = pp.tile([128, 256], f32, tag="ws", bufs=2,
                                          name=f"scT{b}_{qp}_{h}_{kc}")
                            nc.tensor.matmul(
                                scT,
                                qkT[hsl, 1, ts(kc, 128)],
                                qkT[hsl, 0, ts(qp, 256)],
                                start=True, stop=True)
                            if kc == nkc - 2:
                                nc.vector.tensor_add(scT, scT, cm2a_sb)
                            elif kc == nkc - 1:
                                nc.vector.tensor_add(scT, scT, cm2b_sb)
                            exk = sp.tile([128, 256], bf16, tag="exk", bufs=4,
                                          name=f"exk{b}_{qp}_{h}_{kc}")
                            nc.scalar.activation(exk, scT, AF.Exp, scale=SCALE)
                            pvmm = nc.tensor.matmul(
                                pvt[hsl, 0:256], v_sb[:, kc, hsl], exk,
                                start=(kc == 0), stop=(kc == nkc - 1))
                            if kc == 0:
                                pvfirst = pvmm
                            if kc == nkc - 1 and pvden is not None:
                                dep(pvmm, pvden, "pv stop after last den")
                            dmm = nc.tensor.matmul(
                                pvt[dsl, 256:512], ones128k_sb, exk,
                                start=False, stop=False, skip_group_check=True)
                            if kc == 0:
                                dep(dmm, pvfirst, "den zero after pv start")
                            pvden = dmm
                    for h in range(HPC):
                        hsl = slice(64 * h, 64 * h + 64)
                        dsl = slice(64 * h, 64 * h + 1)
                        rec_sb = sp.tile([128, 256], f32, tag="rec_sb",
                                         bufs=2, name=f"rcs{b}_{qp}_{h}")
                        nc.vector.reciprocal(rec_sb[dsl, :],
                                             pvt[dsl, 256:512])
                        rec_bf = sp.tile([128, 256], bf16, tag="rec_bf",
                                         bufs=2, name=f"rcb{b}_{qp}_{h}")
                        nc.vector.tensor_copy(rec_bf[dsl, :], rec_sb[dsl, :])
                        rb = pp.tile([128, 256], f32, tag="ws", bufs=2,
                                     name=f"rb{b}_{qp}_{h}")
                        nc.tensor.matmul(rb[hsl, :], onesB_sb[dsl, :],
                                         rec_bf[dsl, :],
                                         start=True, stop=True)
                        rb_sb = sp.tile([128, 256], f32, tag="rb_sb",
                                        bufs=2, name=f"rbs{b}_{qp}_{h}")
                        nc.scalar.copy(rb_sb[hsl, :], rb[hsl, :])
                        nc.vector.tensor_mul(oTh[hsl, ts(qp, 256)],
                                             pvt[hsl, 0:256], rb_sb[hsl, :])
                    if qp % 2 == 1:
                        nch = qp // 2
                        for mc in range(DC):
                            wo_ps = pp.tile([128, 512], f32, tag="ws", bufs=2,
                                            name=f"wops{b}_{mc}_{nch}")
                            nc.tensor.matmul(wo_ps, wout_sb[:, mc, :],
                                             oTh[:, ts(nch, 512)],
                                             start=True, stop=True)
                            wo_bf = sp.tile([128, 512], bf16, tag="wo_bf",
                                            bufs=2, name=f"wobf{b}_{mc}_{nch}")
                            nc.vector.tensor_copy(wo_bf, wo_ps)
                            nc.sync.dma_start(arin[b][nch][mc, :, :], wo_bf)
                        nc.gpsimd.collective_compute(
                            "AllReduce", OP.add, replica_groups=rg,
                            ins=[arin[b][nch].opt()],
                            outs=[arout[b][nch].opt()])


            def emit_moe(b):
                oTx = []
                for x in range(2):
                    t = sp.tile([128, DC, 512], bf16, tag="oT", bufs=2,
                                name=f"oT{b}_{x}")
                    for d in range(DC):
                        nc.sync.dma_start(t[:, d, :], arout[b][x][d])
                    oTx.append(t)
                pass
                # ---- gates ----
                g_bc = sp.tile([128, S], bf16, tag="g_bc", bufs=1,
                               name=f"gbc{b}")
                for nch in range(2):
                    lg = pp.tile([E, 512], f32, tag="ws", bufs=2,
                                 name=f"lg{b}_{nch}")
                    for d in range(DC):
                        nc.tensor.matmul(lg, gatew_sb[:, d, :],
                                         oTx[nch][:, d, :],
                                         start=(d == 0), stop=(d == DC - 1))
                    expT = sp.tile([E, 512], bf16, tag="expT",
                                   name=f"expT{b}_{nch}")
                    nc.scalar.activation(expT, lg, AF.Exp, bias=gateb_sb,
                                         scale=1.0)
                    den_ps = pp.tile([1, 512], f32, tag="ws", bufs=2,
                                     name=f"denps{b}_{nch}")
                    nc.tensor.matmul(den_ps, ones8_sb, expT,
                                     start=True, stop=True)
                    sel_ps = pp.tile([1, 512], f32, tag="ws", bufs=2,
                                     name=f"selps{b}_{nch}")
                    nc.tensor.matmul(sel_ps, onehot_sb, expT,
                                     start=True, stop=True)
                    den_sb = sp.tile([1, 512], bf16, tag="den_sb",
                                     name=f"densb{b}_{nch}")
                    nc.scalar.copy(den_sb, den_ps)
                    sel_sb = sp.tile([1, 512], bf16, tag="sel_sb",
                                     name=f"selsb{b}_{nch}")
                    nc.scalar.copy(sel_sb, sel_ps)
                    # broadcast denominator and selected-expert rows to 128
                    # partitions via K=1 matmuls, then g = sel * (1/den)
                    gbd = pp.tile([128, 512], f32, tag="ws", bufs=2,
                                  name=f"gbd{b}_{nch}")
                    nc.tensor.matmul(gbd, onesA_sb[0:1, :], den_sb,
                                     start=True, stop=True)
                    gbs = pp.tile([128, 512], f32, tag="ws", bufs=2,
                                  name=f"gbs{b}_{nch}")
                    nc.tensor.matmul(gbs, onesA_sb[0:1, :], sel_sb,
                                     start=True, stop=True)
                    recd = sp.tile([128, 512], f32, tag="recd", bufs=1,
                                   name=f"recd{b}_{nch}")
                    nc.vector.reciprocal(recd, gbd)
                    nc.vector.tensor_mul(g_bc[:, ts(nch, 512)], gbs, recd)
                # ---- FFN expert ----
                for tt in range(NTT):
                    y_ps = pp.tile([128, DC, TT], f32, tag="y", bufs=1,
                                   name=f"yps{b}_{tt}")

                    def emit_h(hc, tt=tt):
                        hp = pp.tile([128, TT], f32, tag="h", bufs=1,
                                     name=f"hps{b}_{tt}_{hc}")
                        for d in range(DC):
                            nc.tensor.matmul(
                                hp,
                                w1_sb[:, d, ts(hc, 128)],
                                oTx[tt // 2][:, d, ts(tt % 2, TT)],
                                start=(d == 0), stop=(d == DC - 1))
                        return hp

                    hp = emit_h(0)
                    for hc in range(HC):
                        hT = sp.tile([128, TT], bf16, tag="hT", bufs=4,
                                     name=f"hT{b}_{tt}_{hc}")
                        nc.scalar.activation(hT, hp, AF.Gelu_apprx_tanh,
                                             bias=b1_sb[:, hc:hc + 1],
                                             scale=1.0)
                        if hc + 1 < HC:
                            hp = emit_h(hc + 1)
                        for d2 in range(DC):
                            ymm = nc.tensor.matmul(
                                y_ps[:, d2, :],
                                w2_sb[:, hc, ts(d2, 128)],
                                hT,
                                start=(hc == 0 and d2 % 2 == 0),
                                stop=(hc == HC - 1 and d2 % 2 == 1),
                                skip_group_check=(d2 % 2 == 1))
                            if hc == 0 and d2 % 2 == 0:
                                ylast = ymm
                            elif hc == 0 and d2 % 2 == 1:
                                dep(ymm, ylast, "y odd zero after even start")
                            if hc == HC - 1 and d2 % 2 == 0:
                                ylast = ymm
                            elif hc == HC - 1 and d2 % 2 == 1:
                                dep(ymm, ylast, "y stop after even last")
                    for d2 in range(DC):
                        y_bf = sp.tile([128, TT], bf16, tag="y_bf", bufs=3,
                                       name=f"ybf{b}_{tt}_{d2}")
                        nc.vector.scalar_tensor_tensor(
                            y_bf, y_ps[:, d2, :], b2_sb[:, d2:d2 + 1],
                            g_bc[:, ts(tt, TT)],
                            op0=OP.add, op1=OP.mult)
                        nc.sync.dma_start(
                            rsin[b][tt // 2][d2, :, ts(tt % 2, TT)], y_bf)
                    if tt % 2 == 1:
                        x = tt // 2
                        nc.gpsimd.collective_compute(
                            "ReduceScatter", OP.add, replica_groups=rg,
                            ins=[rsin[b][x].opt()], outs=[rsout[b][x].opt()])
                        ob_bf = sp.tile([128, 512], bf16, tag="ob_bf", bufs=1,
                                        name=f"obbf{b}_{x}")
                        nc.sync.dma_start(ob_bf, rsout[b][x][:, :])
                        ob = sp.tile([128, 512], f32, tag="ob", bufs=1,
                                     name=f"ob{b}_{x}")
                        nc.vector.tensor_copy(ob, ob_bf)
                        nc.sync.dma_start(out_e[b, :, ts(x, 512)], ob)
                pass

            # software-pipelined emission: attn runs one batch ahead of moe
            if debug_attn:
                emit_attention(0)
            else:
                emit_attention(0)
                load_moe_weights()
                for b in range(1, B):
                    emit_attention(b)
                    emit_moe(b - 1)
                emit_moe(B - 1)

    nc.compile()
    return nc


def _prep_inputs(inputs):
    """Host-side shard prep: slice/transpose/cast per core."""
    f32 = np.float32

    def b(x):
        return np.ascontiguousarray(x).astype(bf16np)

    x = inputs["x"].astype(f32)
    w_qkv = inputs["w_qkv"].astype(f32)
    w_out = inputs["w_out"].astype(f32)
    qn_g, kn_g = inputs["qn_g"].astype(f32), inputs["kn_g"].astype(f32)
    gate_w, gate_b = inputs["gate_w"].astype(f32), inputs["gate_b"].astype(f32)
    w1, b1, w2, b2 = (inputs["w1"].astype(f32), inputs["b1"].astype(f32),
                      inputs["w2"].astype(f32), inputs["b2"].astype(f32))

    xT = b(x.transpose(0, 2, 1).reshape(B, DC, 128, S))
    gatewr = b(gate_w.reshape(DC, 128, E))
    ii, jj = np.meshgrid(np.arange(128), np.arange(256), indexing="ij")
    cmaskT2a = np.where(ii <= jj, 0.0, NEG).astype(f32)   # diag on left half
    cmaskT2b = np.where(jj < 128, NEG,
                        np.where(ii <= jj - 128, 0.0, NEG)).astype(f32)
    ii, jj = np.meshgrid(np.arange(128), np.arange(128), indexing="ij")
    mmean = b(np.where(ii // 64 == jj // 64, 1.0 / 64, 0.0).astype(f32))
    onesA = np.ones((2, 128), bf16np)
    qg = np.tile(qn_g, 2).reshape(128, 1).astype(f32)
    kg = np.tile(kn_g, 2).reshape(128, 1).astype(f32)
    gateb = gate_b.reshape(E, 1).astype(f32)

    in_maps = []
    for c in range(NC):
        h0, h1 = HPC * c, HPC * c + 1
        cs = np.r_[h0 * 64:(h0 + 1) * 64, h1 * 64:(h1 + 1) * 64]
        wqk_c = np.concatenate(
            [w_qkv[:, cs], w_qkv[:, DIM + cs]], axis=1)          # [1024,256]
        wv_c = w_qkv[:, 2 * DIM + cs]                            # [1024,128]
        wout_c = w_out[cs, :]                                    # [128,1024]
        onehot = np.zeros((E, 1), bf16np)
        onehot[c, 0] = 1.0
        in_maps.append({
            "xT": xT,
            "wqk": b(wqk_c.reshape(DC, 128, 256)),
            "wv": b(wv_c.reshape(DC, 128, 128)),
            "wout": b(wout_c.reshape(128, DC, 128).transpose(1, 0, 2)),
            "w1": b(w1[c].reshape(DC, 128, HID)),
            "w2": b(w2[c].reshape(HC, 128, DIM)),
            "gatew": gatewr,
            "b1s": np.ascontiguousarray(b1[c].reshape(HC, 128).T).astype(f32),
            "b2s": np.ascontiguousarray(b2[c].reshape(DC, 128).T).astype(f32),
            "gateb": gateb,
            "qg": qg,
            "kg": kg,
            "cmaskT2a": cmaskT2a,
            "cmaskT2b": cmaskT2b,
            "ones128k": np.ones((128, 1), bf16np),
            "onesB": np.ones((128, 64), bf16np),
            "mmean": mmean,
            "ones8": np.ones((E, 1), bf16np),
            "onehot": onehot,
            "onesA": onesA,
        })
    return in_maps


def kernel(**inputs):
    from concourse.bass_utils import run_bass_kernel_spmd

    if "nc" not in _cache:
        _cache["nc"] = _build()
    nc = _cache["nc"]
    in_maps = _prep_inputs(inputs)
    res = run_bass_kernel_spmd(nc, in_maps, core_ids=list(range(NC)))
    full = np.empty((B, S, DIM), np.float32)
    for c in range(NC):
        full[:, :, 128 * c:128 * (c + 1)] = (
            res.results[c]["out"].transpose(0, 2, 1))
    return full
